# revision 47
# baseline (speedup 1.0000x reference)
# Multi-head self-attention with RoPE on 8 Trainium2 NeuronCores.
#
# Sharding: batch x head-group. Core c handles batch b = c//4 and heads
# 4*(c%4) .. 4*(c%4)+3 (4 of 16 heads), organized as 2 "pr" pairs of 2
# heads. Each core computes Q/K/V projections for its heads from the full
# (transposed) x[b], runs attention, and produces a partial output
# projection Y_partial = O_core^T.T @ Wo[rows-of-its-heads] in bf16. The
# host sums the four partials per batch (in f32) and adds the bias terms.
#
# Everything on-device is bf16 (inputs, Q/K/V, P=exp(S), O, Wo, Y) with
# f32 PSUM accumulation; measured end-to-end rel err ~7e-3 vs the f32
# reference (budget 2e-2). The softmax scale (1/8) is baked into Wq/qb
# host-side so the exp runs with scale=1.0.
#
# The kernel is ACT(exp)-throughput-bound: the Scalar/ACT engine streams
# one [128,1024] exp per kc-slot at ~1.11us + ~0.55us fixed instruction
# overhead (measured free-run cadence 1.66us), 128 slots total. The whole
# schedule is built around keeping that stream saturated:
#
#   - flat 128-slot pipeline over units (pr, qt): at slot k the S-pair
#     for slot k+4 is emitted (it only ring-WARs on exp(k+2), never on a
#     PV), then fillers, then PV(k-1). PSUM: exp ring [128,2048] (2
#     groups) + 2 oacc banks + 2 proj/V/WO banks.
#   - V tiles 4..15, the pr1 projections+ropes, and the output projection
#     (WO) are woven into the slot schedule as "fillers" so the PE stays
#     busy and ramped without ever starving the exp stream.
#   - at each unit boundary the [65,512] O accumulators (row 64 = Z via
#     the ones-column-in-V trick) are evicted raw to SBUF with two fast
#     DVE copies; the softmax normalization (reciprocal_approx_fast +
#     gpsimd partition_broadcast + DVE multiply) runs later, off the
#     critical path. NOTE: custom DVE ops (reciprocal_approx_*) need
#     SBUF operands at partition base 0 - PSUM reads or partition-offset
#     inputs silently corrupt on HW.
#   - input DMAs are staged across both HWDGE rings so the first
#     projection matmul starts ~15us in (x quarter 0 split in halves,
#     first-needed weights first, RoPE tables quarter-interleaved).
#
# RoPE: head-dim rows are pair-interleaved (d' = [0,32,1,33,...]) via a
# host permutation of Wq/Wk columns so the rotate-half partner lives on
# the adjacent partition; a DVE stream_shuffle (pair swap on u32-bitcast
# bf16 pairs) + 2 muls + 1 add apply the rotation per 512-col t-quarter.

import os
import sys

import numpy as np

try:
    import ml_dtypes

    BF16 = np.dtype(ml_dtypes.bfloat16)
except ImportError:  # pragma: no cover
    BF16 = None

for _p in ("/opt/trn_rl_repo", os.path.expanduser("~/.axon_site/_ro/trn_rl_repo")):
    if os.path.isdir(_p) and _p not in sys.path:
        sys.path.insert(0, _p)

B, T, D = 2, 2048, 1024
NHEADS, HD, HALF = 16, 64, 32
HPC = 4  # heads per core
N_CORES = 8
ROPE_BASE = 10000.0
SCALE = float(HD) ** -0.5  # 0.125
NDC = D // 128  # 8 contraction chunks for the projections
NKC = T // 128  # 16 k chunks per head

_SHUF_MASK = [i ^ 1 for i in range(32)]

_ctx: dict = {}


def _build_nc(iters: int = 0, phase: str = "full"):
    import concourse.bacc as bacc
    import concourse.mybir as mybir
    import concourse.tile as tile

    f32 = mybir.dt.float32
    bf16 = mybir.dt.bfloat16
    u32 = mybir.dt.uint32
    i32 = mybir.dt.int32
    Exp = mybir.ActivationFunctionType.Exp
    MUL = mybir.AluOpType.mult

    nc = bacc.Bacc("TRN2", target_bir_lowering=False, debug=False)

    # packed inputs (see _host_inputs for layouts)
    xq_d = nc.dram_tensor("xqp", [128, 4 * NDC * 512], bf16, kind="ExternalInput").ap()
    wqk_d = nc.dram_tensor("wqk", [128, 4096], bf16, kind="ExternalInput").ap()
    wvp_d = nc.dram_tensor("wvp", [128, 2048], bf16, kind="ExternalInput").ap()
    tbl_d = nc.dram_tensor("tbl", [128, 4096], bf16, kind="ExternalInput").ap()
    bia_d = nc.dram_tensor("bia", [128, 4], f32, kind="ExternalInput").ap()
    wo_d = nc.dram_tensor("wop", [128, 2048], bf16, kind="ExternalInput").ap()
    y_d = nc.dram_tensor("y", [T, D], bf16, kind="ExternalOutput").ap()
    y_r = y_d.rearrange("(b p) e -> p b e", p=128)  # [128, 16, 1024]

    with tile.TileContext(nc) as tc:
        with (
            tc.tile_pool(name="xpool", bufs=4) as xpool,
            tc.tile_pool(name="wpool", bufs=1) as wpool,
            tc.tile_pool(name="qkpool", bufs=4) as qkpool,
            tc.tile_pool(name="shpool", bufs=2) as shpool,
            tc.tile_pool(name="vpool", bufs=16) as vpool,
            tc.tile_pool(name="ppool", bufs=8) as ppool,
            tc.tile_pool(name="otpool", bufs=2) as otpool,
            tc.tile_pool(name="ypool", bufs=2) as ypool,
            tc.tile_pool(name="rzpool", bufs=4) as rzpool,
            tc.tile_pool(name="rbpool", bufs=4) as rbpool,
            tc.tile_pool(name="pring", bufs=1, space="PSUM") as pring,
            tc.tile_pool(name="pso", bufs=2, space="PSUM") as pso,
            tc.tile_pool(name="psw", bufs=2, space="PSUM") as psw,
        ):

            def body():
                # ---- staged input DMAs across both HWDGE rings ----
                # sync:   bia, wq0, wk0, xq0b, wvp, xq2, wqk[2048:], wo, y out
                # scalar: xq0a, tbl, xq1, xq3, y out (tail)
                bia_t = wpool.tile([128, 4], f32, tag="bia", name="bia_t")
                nc.sync.dma_start(out=bia_t[:], in_=bia_d)
                wqk_t = wpool.tile([128, 4096], bf16, tag="wqk", name="wqk_t")
                nc.sync.dma_start(out=wqk_t[:, 0:2048], in_=wqk_d[:, 0:2048])
                # x quarter 0 split into chunk-halves across both rings
                xq0a = xpool.tile([128, 2048], bf16, tag="xh", name="xq0a")
                nc.scalar.dma_start(out=xq0a[:], in_=xq_d[:, 0:2048])
                xq0b = xpool.tile([128, 2048], bf16, tag="xh", name="xq0b")
                nc.sync.dma_start(out=xq0b[:], in_=xq_d[:, 2048:4096])
                xqt = [None]
                for q in range(1, 4):
                    xq = xpool.tile([128, NDC * 512], bf16, tag="x", name=f"xq{q}")
                    xqt.append(xq)
                nc.scalar.dma_start(out=xqt[1][:], in_=xq_d[:, 4096:8192])
                tbl_t = wpool.tile([128, 4096], bf16, tag="tbl", name="tbl_t")
                nc.sync.dma_start(out=tbl_t[:, 0:2048], in_=tbl_d[:, 0:2048])
                wv_t = wpool.tile([128, 2048], bf16, tag="wv", name="wv_t")
                nc.sync.dma_start(out=wv_t[:], in_=wvp_d)
                nc.scalar.dma_start(out=tbl_t[:, 2048:4096], in_=tbl_d[:, 2048:4096])
                nc.sync.dma_start(out=xqt[2][:], in_=xq_d[:, 8192:12288])
                nc.scalar.dma_start(out=xqt[3][:], in_=xq_d[:, 12288:16384])
                nc.sync.dma_start(out=wqk_t[:, 2048:4096], in_=wqk_d[:, 2048:4096])
                wo_t = wpool.tile([128, 2048], bf16, tag="wo", name="wo_t")
                nc.sync.dma_start(out=wo_t[:], in_=wo_d)

                def xsl(q, ch, lo, hi):  # x slice for quarter q, chunk ch
                    if q == 0:
                        t_ = xq0a if ch < 4 else xq0b
                        return t_[:, (ch % 4) * 512 + lo : (ch % 4) * 512 + hi]
                    return xqt[q][:, ch * 512 + lo : ch * 512 + hi]

                ring = pring.tile([128, 2048], f32, tag="ring", name="ring")

                # ---- Q/K projections; eviction = DVE tensor_scalar (+bias) ----
                def project(pr, is_k, dst, q):
                    base = pr * 2048 + (1024 if is_k else 0)
                    ps = psw.tile([128, 512], f32, tag="pw", name=f"ps_{pr}{is_k}{q}")
                    for ch in range(NDC):
                        nc.tensor.matmul(
                            ps[:],
                            wqk_t[:, base + ch * 128 : base + (ch + 1) * 128],
                            xsl(q, ch, 0, 512),
                            start=(ch == 0),
                            stop=(ch == NDC - 1),
                        )
                    nc.vector.tensor_scalar_add(
                        dst[:, q * 512 : (q + 1) * 512],
                        ps[:],
                        bia_t[:, is_k * 2 + pr : is_k * 2 + pr + 1],
                    )

                def rope(t_, name, q):  # rope one 512-col t-quarter in place
                    # tbl layout is quarter-interleaved: [cos_q | sin_q] x 4
                    sl = slice(q * 512, (q + 1) * 512)
                    cos_q = tbl_t[:, q * 1024 : q * 1024 + 512]
                    sin_q = tbl_t[:, q * 1024 + 512 : (q + 1) * 1024]
                    sh = shpool.tile([128, 512], bf16, tag="sh", name=f"sh_{name}{q}")
                    nc.vector.stream_shuffle(
                        sh.bitcast(u32)[:], t_.bitcast(u32)[:, q * 256 : (q + 1) * 256],
                        _SHUF_MASK,
                    )
                    nc.vector.tensor_tensor(
                        out=t_[:, sl], in0=t_[:, sl], in1=cos_q, op=MUL
                    )
                    nc.vector.tensor_tensor(out=sh[:], in0=sh[:], in1=sin_q, op=MUL)
                    nc.vector.tensor_tensor(
                        out=t_[:, sl], in0=t_[:, sl], in1=sh[:], op=mybir.AluOpType.add
                    )

                qts, kts = [], []
                for pr in range(2):
                    qts.append(qkpool.tile([128, T], bf16, tag="qk", name=f"qt{pr}"))
                    kts.append(qkpool.tile([128, T], bf16, tag="qk", name=f"kt{pr}"))

                # ---- V projection (bf16 tiles, ones col via memset) ----
                vts = [None] * NKC

                def vproj(tk):
                    vt = vpool.tile([128, HPC * 65], bf16, tag="v", name=f"v{tk}")
                    nc.vector.memset(
                        vt.rearrange("p (h c) -> p h c", c=65)[:, :, 64:65], 1.0
                    )
                    ps = psw.tile([128, 256], f32, tag="pw", name=f"psv{tk}")
                    for ch in range(NDC):
                        nc.tensor.matmul(
                            ps[:],
                            xsl(tk // 4, ch, (tk % 4) * 128, (tk % 4) * 128 + 128),
                            wv_t[:, ch * 256 : (ch + 1) * 256],
                            start=(ch == 0),
                            stop=(ch == NDC - 1),
                        )
                    nc.vector.tensor_copy(
                        vt.rearrange("p (h c) -> p h c", c=65)[:, :, 0:64],
                        ps.rearrange("p (h c) -> p h c", c=64),
                    )
                    vts[tk] = vt

                ot0 = otpool.tile([128, T], bf16, tag="o", name="ot0")
                ot1 = otpool.tile([128, T], bf16, tag="o", name="ot1")
                ots = [ot0, ot1]

                # ---- flat attention pipeline over 128 global kc-slots ----
                # slot s -> unit u = s//16 = (pr = u//4, qt qi = u%4), kc = s%16.
                # Emission: prologue S(0..3); then slot k: [S(k+4)] [filler]
                # [PV(k)]; exp(s) is emitted right after S(s). The 4-slot
                # S-lead keeps the ACT exp stream saturated: S(k+4) only
                # ring-waits exp(k+2), never a PV.
                NSLOT = 128
                pts = [None] * NSLOT
                oaccs = {}

                def emit_S(s):
                    u, kc = s // 16, s % 16
                    pr, qi = u // 4, u % 4
                    goff = (s % 2) * 1024
                    for hh in range(2):
                        nc.tensor.matmul(
                            ring[:, goff + hh * 512 : goff + (hh + 1) * 512],
                            kts[pr][hh * 64 : hh * 64 + 64, kc * 128 : (kc + 1) * 128],
                            qts[pr][hh * 64 : hh * 64 + 64, qi * 512 : qi * 512 + 512],
                            start=True,
                            stop=True,
                        )
                    # softmax scale is baked into Wq host-side; plain exp here
                    pt = ppool.tile([128, 1024], bf16, tag="p", name=f"p_{s}")
                    nc.scalar.activation(
                        pt[:], ring[:, goff : goff + 1024], Exp, bias=0.0, scale=1.0
                    )
                    pts[s] = pt

                def emit_PV(s):
                    u, kc = s // 16, s % 16
                    pr, qi = u // 4, u % 4
                    if kc == 0:
                        oaccs[u] = [
                            pso.tile([65, 512], f32, tag="oa", name=f"o_{u}_{hh}")
                            for hh in range(2)
                        ]
                    for hh in range(2):
                        nc.tensor.matmul(
                            oaccs[u][hh][:],
                            vts[kc][:, (2 * pr + hh) * 65 : (2 * pr + hh + 1) * 65],
                            pts[s][:, hh * 512 : (hh + 1) * 512],
                            start=(kc == 0),
                            stop=(kc == NKC - 1),
                        )
                    pts[s] = None

                # raw O eviction: one fast DVE copy per hh frees the oacc
                # PSUM bank ~0.4us after the unit's last PV; the normalize
                # chain then runs entirely in SBUF, off the critical path.
                raws = {}

                def raw_copy(u):
                    raw = rzpool.tile([64, 2048], f32, tag="rz", name=f"raw_{u}")
                    # Z rows land in a separate partition-base-0 tile: the
                    # custom DVE reciprocal requires base-0 operands
                    zr = rzpool.tile([1, 1024], f32, tag="zr", name=f"zr_{u}")
                    for hh in range(2):
                        nc.vector.tensor_copy(
                            raw[:, hh * 512 : (hh + 1) * 512], oaccs[u][hh][0:64, :]
                        )
                        nc.vector.tensor_copy(
                            zr[0:1, hh * 512 : (hh + 1) * 512], oaccs[u][hh][64:65, :]
                        )
                    raws[u] = (raw, zr)

                def normalize(u):
                    pr, qi = u // 4, u % 4
                    qs = qi * 512
                    raw, zr = raws[u]
                    for hh in range(2):
                        hs = slice(hh * 512, (hh + 1) * 512)
                        rz = rbpool.tile([1, 512], f32, tag="rz", name=f"rzz_{u}_{hh}")
                        nc.vector.reciprocal_approx_fast(rz[0:1, :], zr[0:1, hs])
                        rc = rbpool.tile([64, 512], f32, tag="rb", name=f"rc_{u}_{hh}")
                        nc.gpsimd.partition_broadcast(
                            rc[:, :], rz[0:1, :], channels=64
                        )
                        nc.vector.tensor_tensor(
                            out=ots[pr][hh * 64 : hh * 64 + 64, qs : qs + 512],
                            in0=raw[0:64, hs],
                            in1=rc[:, :],
                            op=MUL,
                        )

                # wo sub-unit j of qt qi: t-chunk tt = 4*qi+j (2 matmul pairs
                # + casts); qi<3 packs 2 chunks per [128,2048] ysb + 1 DMA,
                # the tail qt (qi=3) DMAs each chunk separately.
                ysbs = {}

                def wo_sub(qi, j):
                    tt = 4 * qi + j
                    if j % 2 == 0:
                        ysbs[(qi, j // 2)] = ypool.tile(
                            [128, 2048], bf16, tag="y", name=f"y_{qi}_{j // 2}"
                        )
                    ysb = ysbs[(qi, j // 2)]
                    for eh in range(2):
                        # tail qt borrows the (free) oacc pool for every other
                        # accumulator so the cast evictions don't serialize it
                        pool = pso if qi == 3 and eh == 1 else psw
                        tag = "oa" if pool is pso else "pw"
                        yps = pool.tile([128, 512], f32, tag=tag, name=f"yp_{tt}_{eh}")
                        for r in range(2):
                            nc.tensor.matmul(
                                yps[:],
                                ots[r][:, tt * 128 : (tt + 1) * 128],
                                wo_t[:, r * 1024 + eh * 512 : r * 1024 + (eh + 1) * 512],
                                start=(r == 0),
                                stop=(r == 1),
                            )
                        nc.vector.tensor_copy(
                            ysb[:, (j % 2) * 1024 + eh * 512 : (j % 2) * 1024 + (eh + 1) * 512],
                            yps[:],
                        )
                    if qi == 3:
                        eng = nc.sync if j % 2 == 0 else nc.scalar
                        eng.dma_start(
                            out=y_r[:, tt : tt + 1, :],
                            in_=ysb.rearrange("p (b e) -> p b e", e=1024)[
                                :, j % 2 : j % 2 + 1, :
                            ],
                        )
                    elif j % 2 == 1:
                        nc.sync.dma_start(
                            out=y_r[:, 4 * qi + j - 1 : 4 * qi + j + 1, :],
                            in_=ysb.rearrange("p (b e) -> p b e", e=1024),
                        )

                # ---- filler schedule: slot -> list of thunks; fillers are
                # emitted BEFORE the slot's S so late Q/K quarters can feed
                # the S stream just-in-time ----
                fillers = {}

                def add_filler(slot, fn):
                    fillers.setdefault(slot, []).append(fn)

                def qk_part(q, is_k):  # one Q0/K0 proj quarter + its rope
                    def fn():
                        t_ = kts[0] if is_k else qts[0]
                        project(0, is_k, t_, q)
                        rope(t_, "k0" if is_k else "q0", q)

                    return fn

                # quarter q roped before S(4q) at slot 4q-4; split Q/K halves
                # across two slots to limit per-slot PE insertion
                for q, (sq, sk) in ((1, (0, 0)), (2, (2, 3)), (3, (6, 7))):
                    add_filler(sq, qk_part(q, 0))
                    add_filler(sk, qk_part(q, 1))
                vslots = [1, 4, 4, 5, 8, 8, 9, 10, 11, 12, 13, 14]  # V4..V15
                for i, sl in enumerate(vslots):
                    add_filler(sl, (lambda tk: lambda: vproj(tk))(4 + i))
                pj = 0  # pr1 projections + quarter-ropes woven into units 2-3
                for is_k in range(2):
                    for q in range(4):
                        add_filler(
                            32 + 2 * pj,
                            (lambda ik, qq: lambda: project(
                                1, ik, kts[1] if ik else qts[1], qq
                            ))(is_k, q),
                        )
                        add_filler(
                            33 + 2 * pj,
                            (lambda ik, qq: lambda: rope(
                                kts[1] if ik else qts[1], "k1" if ik else "q1", qq
                            ))(is_k, q),
                        )
                        pj += 1
                # wo(qi) woven into unit 5+qi AFTER each slot's S/PV so it
                # never delays the exp stream; wo(3) is the tail
                postfill = {}
                for qi in range(3):
                    for j in range(4):
                        postfill.setdefault(
                            (5 + qi) * 16 + 2 + 3 * j, []
                        ).append((lambda a, b: lambda: wo_sub(a, b))(qi, j))

                # ---- prologue: warm the PE p-state on the (tiny, landed)
                # bias tile while x/weights stream in, then quarter 0 ----
                warm = psw.tile([128, 512], f32, tag="pw", name="warm")
                for w in range(24):
                    nc.tensor.matmul(
                        warm[0:4, 0:4],
                        bia_t.bitcast(bf16)[:, 0:4],
                        bia_t.bitcast(bf16)[:, 0:4],
                        start=True,
                        stop=True,
                    )
                qk_part(0, 0)()
                qk_part(0, 1)()
                for s in range(4):
                    emit_S(s)
                for tk in range(4):
                    vproj(tk)

                # ---- steady state; PV lags S-emission by 5 slots ----
                for k in range(NSLOT + 1):
                    if k > 16 and k % 16 == 1:
                        normalize(k // 16 - 1)
                    for fn in fillers.get(k, ()):
                        fn()
                    if k + 4 < NSLOT:
                        emit_S(k + 4)
                    if k > 0:
                        emit_PV(k - 1)
                    if k > 0 and k % 16 == 0:
                        raw_copy(k // 16 - 1)
                    for fn in postfill.get(k, ()):
                        fn()

                # ---- tail: wo for the last qt ----
                normalize(7)
                for j in range(4):
                    wo_sub(3, j)

            if iters:
                import concourse.mybir as _mb

                with tc.For_i(
                    0,
                    iters,
                    1,
                    hint_engines=(
                        _mb.EngineType.PE,
                        _mb.EngineType.Activation,
                        _mb.EngineType.DVE,
                        _mb.EngineType.SP,
                        _mb.EngineType.Pool,
                    ),
                    staggered_reset=True,
                ) as _iv:
                    body()
            else:
                body()

    nc.compile()
    return nc


def _host_inputs(x, wq_w, wq_b, wk_w, wk_b, wv_w, wv_b, wo_w, wo_b):
    """Build the 8 per-core input maps (all host-side slicing/packing)."""
    f = np.float32
    x = np.asarray(x, f)
    wq_w = np.asarray(wq_w, f)
    wk_w = np.asarray(wk_w, f)
    wv_w = np.asarray(wv_w, f)
    wo_w = np.asarray(wo_w, f)
    wq_b = np.asarray(wq_b, f)
    wk_b = np.asarray(wk_b, f)
    wv_b = np.asarray(wv_b, f)
    wo_b = np.asarray(wo_b, f)

    def chunkpack(a, ncol):  # [1024, ncol] -> [128, 8*ncol] (D-chunk packed)
        return np.ascontiguousarray(
            a.reshape(NDC, 128, ncol).transpose(1, 0, 2).reshape(128, NDC * ncol)
        )

    # RoPE tables in fp32, mirroring the reference formulas; stored bf16.
    pos = np.arange(T, dtype=f)[:, None]
    idx = np.arange(HALF, dtype=f)[None, :]
    inv_freq = (f(1.0) / (f(ROPE_BASE) ** (idx / f(HALF)))).astype(f)
    ang = pos * inv_freq  # [T, 32]
    cosv, sinv = np.cos(ang).astype(f), np.sin(ang).astype(f)
    cos64 = np.repeat(cosv.T, 2, axis=0)  # [64, T]
    sin64 = np.repeat(sinv.T, 2, axis=0)
    sin64[0::2] *= -1  # rows 2j: -sin, rows 2j+1: +sin
    cos128 = np.tile(cos64, (2, 1))
    sin128 = np.tile(sin64, (2, 1))
    # quarter-interleaved: [cos_q | sin_q] per 512-col t-quarter
    tbl = np.ascontiguousarray(
        np.concatenate(
            [
                np.concatenate(
                    [cos128[:, q * 512 : (q + 1) * 512], sin128[:, q * 512 : (q + 1) * 512]],
                    axis=1,
                )
                for q in range(4)
            ],
            axis=1,
        ).astype(BF16)
    )

    perm64 = np.empty(64, np.int64)
    perm64[0::2] = np.arange(32)
    perm64[1::2] = np.arange(32) + 32

    # x[b]^T quarter-packed: quarter q holds all 8 D-chunks for t in
    # [512q, 512(q+1)): [128, 8ch x 512t]
    xqp = []
    for b in range(B):
        xt = x[b].T.reshape(NDC, 128, 4, 512)  # [ch, p, q, t]
        xqp.append(
            np.ascontiguousarray(
                xt.transpose(2, 1, 0, 3).reshape(4, 128, NDC * 512)
                .transpose(1, 0, 2).reshape(128, 4 * NDC * 512)
            ).astype(BF16)
        )

    in_maps = []
    for c in range(N_CORES):
        b, g = c // 4, c % 4
        heads = np.arange(4 * g, 4 * g + 4)
        v_cols = np.concatenate([np.arange(h * 64, (h + 1) * 64) for h in heads])
        # softmax scale baked into Wq/qb so the exp runs with scale=1.0
        wqk_parts, bia_cols = [], []
        for w_, b_, sc in ((wq_w, wq_b, np.float32(SCALE)), (wk_w, wk_b, np.float32(1.0))):
            for pr in range(2):
                prheads = heads[2 * pr : 2 * pr + 2]
                cols = np.concatenate([h * 64 + perm64 for h in prheads])
                wqk_parts.append((pr, chunkpack(w_[:, cols] * sc, 128)))
                bia_cols.append((pr, b_[cols] * sc))
        # layout: wq0 | wk0 | wq1 | wk1  (each [128, 1024])
        order = [0, 2, 1, 3]  # indices into wqk_parts (built q0,q1,k0,k1)
        wqk = np.concatenate([wqk_parts[i][1] for i in order], axis=1).astype(BF16)
        # bias cols: qb0, qb1, kb0, kb1
        bia = np.stack(
            [bia_cols[0][1], bia_cols[1][1], bia_cols[2][1], bia_cols[3][1]], axis=1
        ).astype(f)
        wvp = chunkpack(wv_w[:, v_cols], 256).astype(BF16)
        wop = np.ascontiguousarray(
            wo_w[v_cols, :]
            .reshape(2, 128, D)
            .transpose(1, 0, 2)
            .reshape(128, 2 * D)
            .astype(BF16)
        )
        in_maps.append(
            {
                "xqp": xqp[b],
                "wqk": np.ascontiguousarray(wqk),
                "wvp": np.ascontiguousarray(wvp),
                "tbl": tbl,
                "bia": np.ascontiguousarray(bia),
                "wop": wop,
            }
        )

    beff = (
        wo_b.astype(np.float64) + wv_b.astype(np.float64) @ wo_w.astype(np.float64)
    ).astype(f)
    return in_maps, beff


def kernel(x, wq_w, wq_b, wk_w, wk_b, wv_w, wv_b, wo_w, wo_b):
    from concourse import bass2jax

    in_maps, beff = _host_inputs(
        x, wq_w, wq_b, wk_w, wk_b, wv_w, wv_b, wo_w, wo_b
    )
    if "nc" not in _ctx:
        _ctx["nc"] = _build_nc(0)
    res = bass2jax.run_bass_via_pjrt(_ctx["nc"], in_maps, n_cores=N_CORES)
    y = np.empty((B, T, D), np.float32)
    for b in range(B):
        acc = np.asarray(res[4 * b]["y"], np.float32)
        for g in range(1, 4):
            acc += np.asarray(res[4 * b + g]["y"], np.float32)
        y[b] = acc + beff[None, :]
    return y


# revision 48
# speedup vs baseline: 1.1519x; 1.1519x over previous
# Multi-head self-attention with RoPE on 8 Trainium2 NeuronCores.
#
# Sharding: batch x head-group. Core c handles batch b = c//4 and heads
# 4*(c%4) .. 4*(c%4)+3 (4 of 16 heads), organized as 2 "pr" pairs of 2
# heads. Each core computes Q/K/V projections for its heads from the full
# (transposed) x[b], runs attention, and produces a partial output
# projection Y_partial = O_core^T.T @ Wo[rows-of-its-heads] in bf16. The
# host sums the four partials per batch (in f32) and adds the bias terms.
#
# Everything on-device is bf16 (inputs, Q/K/V, P=exp(S), O, Wo, Y) with
# f32 PSUM accumulation; measured end-to-end rel err ~7e-3 vs the f32
# reference (budget 2e-2). The softmax scale (1/8) is baked into Wq/qb
# host-side so the exp runs with scale=1.0.
#
# The kernel is ACT(exp)-throughput-bound: the Scalar/ACT engine streams
# one [128,1024] exp per kc-slot at ~1.11us + ~0.55us fixed instruction
# overhead (measured free-run cadence 1.66us), 128 slots total. The whole
# schedule is built around keeping that stream saturated:
#
#   - flat 128-slot pipeline over units (pr, qt): at slot k the S-pair
#     for slot k+4 is emitted (it only ring-WARs on exp(k+2), never on a
#     PV), then fillers, then PV(k-1). PSUM: exp ring [128,2048] (2
#     groups) + 2 oacc banks + 2 proj/V/WO banks.
#   - V tiles 4..15 and the pr1 projections+ropes are woven in as
#     pre-S fillers (they produce data the S stream needs); the output
#     projection (WO) is woven in AFTER each slot's S/PV so its matmuls
#     never delay the exp stream.
#   - at each unit boundary the [65,512] O accumulators (row 64 = Z via
#     the ones-column-in-V trick) are evicted raw to SBUF with two fast
#     DVE copies; the softmax normalization (reciprocal_approx_fast +
#     gpsimd partition_broadcast + DVE multiply) runs later, off the
#     critical path. NOTE: custom DVE ops (reciprocal_approx_*) need
#     SBUF operands at partition base 0 - PSUM reads or partition-offset
#     inputs silently corrupt on HW.
#   - input DMAs are staged across both HWDGE rings so the first
#     projection matmul starts ~15us in (x quarter 0 split in halves,
#     first-needed weights first, RoPE tables quarter-interleaved).
#
# RoPE: head-dim rows are pair-interleaved (d' = [0,32,1,33,...]) via a
# host permutation of Wq/Wk columns so the rotate-half partner lives on
# the adjacent partition; a DVE stream_shuffle (pair swap on u32-bitcast
# bf16 pairs) + 2 muls + 1 add apply the rotation per 512-col t-quarter.

import os
import sys

import numpy as np

try:
    import ml_dtypes

    BF16 = np.dtype(ml_dtypes.bfloat16)
except ImportError:  # pragma: no cover
    BF16 = None

for _p in ("/opt/trn_rl_repo", os.path.expanduser("~/.axon_site/_ro/trn_rl_repo")):
    if os.path.isdir(_p) and _p not in sys.path:
        sys.path.insert(0, _p)

B, T, D = 2, 2048, 1024
NHEADS, HD, HALF = 16, 64, 32
HPC = 4  # heads per core
N_CORES = 8
ROPE_BASE = 10000.0
SCALE = float(HD) ** -0.5  # 0.125
NDC = D // 128  # 8 contraction chunks for the projections
NKC = T // 128  # 16 k chunks per head

_SHUF_MASK = [i ^ 1 for i in range(32)]

_ctx: dict = {}


def _build_nc(iters: int = 0, phase: str = "full"):
    import concourse.bacc as bacc
    import concourse.mybir as mybir
    import concourse.tile as tile

    f32 = mybir.dt.float32
    bf16 = mybir.dt.bfloat16
    u32 = mybir.dt.uint32
    i32 = mybir.dt.int32
    Exp = mybir.ActivationFunctionType.Exp
    MUL = mybir.AluOpType.mult

    nc = bacc.Bacc("TRN2", target_bir_lowering=False, debug=False)

    # packed inputs (see _host_inputs for layouts)
    xq_d = nc.dram_tensor("xqp", [128, 4 * NDC * 512], bf16, kind="ExternalInput").ap()
    wqk_d = nc.dram_tensor("wqk", [128, 4096], bf16, kind="ExternalInput").ap()
    wvp_d = nc.dram_tensor("wvp", [128, 2048], bf16, kind="ExternalInput").ap()
    tbl_d = nc.dram_tensor("tbl", [128, 4096], bf16, kind="ExternalInput").ap()
    bia_d = nc.dram_tensor("bia", [128, 4], f32, kind="ExternalInput").ap()
    wo_d = nc.dram_tensor("wop", [128, 2048], bf16, kind="ExternalInput").ap()
    y_d = nc.dram_tensor("y", [T, D], bf16, kind="ExternalOutput").ap()
    y_r = y_d.rearrange("(b p) e -> p b e", p=128)  # [128, 16, 1024]

    with tile.TileContext(nc) as tc:
        with (
            tc.tile_pool(name="xpool", bufs=4) as xpool,
            tc.tile_pool(name="wpool", bufs=1) as wpool,
            tc.tile_pool(name="qkpool", bufs=4) as qkpool,
            tc.tile_pool(name="shpool", bufs=2) as shpool,
            tc.tile_pool(name="vpool", bufs=16) as vpool,
            tc.tile_pool(name="ppool", bufs=8) as ppool,
            tc.tile_pool(name="otpool", bufs=2) as otpool,
            tc.tile_pool(name="ypool", bufs=2) as ypool,
            tc.tile_pool(name="rzpool", bufs=4) as rzpool,
            tc.tile_pool(name="rbpool", bufs=4) as rbpool,
            tc.tile_pool(name="pring", bufs=1, space="PSUM") as pring,
            tc.tile_pool(name="pso", bufs=2, space="PSUM") as pso,
            tc.tile_pool(name="psw", bufs=2, space="PSUM") as psw,
        ):

            def body():
                # ---- staged input DMAs across both HWDGE rings ----
                # sync:   bia, wq0, wk0, xq0b, wvp, xq2, wqk[2048:], wo, y out
                # scalar: xq0a, tbl, xq1, xq3, y out (tail)
                bia_t = wpool.tile([128, 4], f32, tag="bia", name="bia_t")
                nc.sync.dma_start(out=bia_t[:], in_=bia_d)
                wqk_t = wpool.tile([128, 4096], bf16, tag="wqk", name="wqk_t")
                nc.sync.dma_start(out=wqk_t[:, 0:2048], in_=wqk_d[:, 0:2048])
                # x quarter 0 split into chunk-halves across both rings
                xq0a = xpool.tile([128, 2048], bf16, tag="xh", name="xq0a")
                nc.scalar.dma_start(out=xq0a[:], in_=xq_d[:, 0:2048])
                xq0b = xpool.tile([128, 2048], bf16, tag="xh", name="xq0b")
                nc.sync.dma_start(out=xq0b[:], in_=xq_d[:, 2048:4096])
                xqt = [None]
                for q in range(1, 4):
                    xq = xpool.tile([128, NDC * 512], bf16, tag="x", name=f"xq{q}")
                    xqt.append(xq)
                nc.scalar.dma_start(out=xqt[1][:], in_=xq_d[:, 4096:8192])
                tbl_t = wpool.tile([128, 4096], bf16, tag="tbl", name="tbl_t")
                nc.sync.dma_start(out=tbl_t[:, 0:2048], in_=tbl_d[:, 0:2048])
                wv_t = wpool.tile([128, 2048], bf16, tag="wv", name="wv_t")
                nc.sync.dma_start(out=wv_t[:], in_=wvp_d)
                nc.scalar.dma_start(out=tbl_t[:, 2048:4096], in_=tbl_d[:, 2048:4096])
                nc.sync.dma_start(out=xqt[2][:], in_=xq_d[:, 8192:12288])
                nc.scalar.dma_start(out=xqt[3][:], in_=xq_d[:, 12288:16384])
                nc.sync.dma_start(out=wqk_t[:, 2048:4096], in_=wqk_d[:, 2048:4096])
                wo_t = wpool.tile([128, 2048], bf16, tag="wo", name="wo_t")
                nc.sync.dma_start(out=wo_t[:], in_=wo_d)

                def xsl(q, ch, lo, hi):  # x slice for quarter q, chunk ch
                    if q == 0:
                        t_ = xq0a if ch < 4 else xq0b
                        return t_[:, (ch % 4) * 512 + lo : (ch % 4) * 512 + hi]
                    return xqt[q][:, ch * 512 + lo : ch * 512 + hi]

                ring = pring.tile([128, 2048], f32, tag="ring", name="ring")

                # ---- Q/K projections; eviction = DVE tensor_scalar (+bias) ----
                def project(pr, is_k, dst, q):
                    base = pr * 2048 + (1024 if is_k else 0)
                    ps = psw.tile([128, 512], f32, tag="pw", name=f"ps_{pr}{is_k}{q}")
                    for ch in range(NDC):
                        nc.tensor.matmul(
                            ps[:],
                            wqk_t[:, base + ch * 128 : base + (ch + 1) * 128],
                            xsl(q, ch, 0, 512),
                            start=(ch == 0),
                            stop=(ch == NDC - 1),
                        )
                    nc.vector.tensor_scalar_add(
                        dst[:, q * 512 : (q + 1) * 512],
                        ps[:],
                        bia_t[:, is_k * 2 + pr : is_k * 2 + pr + 1],
                    )

                def rope(t_, name, q):  # rope one 512-col t-quarter in place
                    # tbl layout is quarter-interleaved: [cos_q | sin_q] x 4
                    sl = slice(q * 512, (q + 1) * 512)
                    cos_q = tbl_t[:, q * 1024 : q * 1024 + 512]
                    sin_q = tbl_t[:, q * 1024 + 512 : (q + 1) * 1024]
                    sh = shpool.tile([128, 512], bf16, tag="sh", name=f"sh_{name}{q}")
                    nc.vector.stream_shuffle(
                        sh.bitcast(u32)[:], t_.bitcast(u32)[:, q * 256 : (q + 1) * 256],
                        _SHUF_MASK,
                    )
                    nc.vector.tensor_tensor(
                        out=t_[:, sl], in0=t_[:, sl], in1=cos_q, op=MUL
                    )
                    nc.vector.tensor_tensor(out=sh[:], in0=sh[:], in1=sin_q, op=MUL)
                    nc.vector.tensor_tensor(
                        out=t_[:, sl], in0=t_[:, sl], in1=sh[:], op=mybir.AluOpType.add
                    )

                qts, kts = [], []
                for pr in range(2):
                    qts.append(qkpool.tile([128, T], bf16, tag="qk", name=f"qt{pr}"))
                    kts.append(qkpool.tile([128, T], bf16, tag="qk", name=f"kt{pr}"))

                # ---- V projection (bf16 tiles, ones col via memset) ----
                vts = [None] * NKC

                def vproj(tk):
                    vt = vpool.tile([128, HPC * 65], bf16, tag="v", name=f"v{tk}")
                    nc.vector.memset(
                        vt.rearrange("p (h c) -> p h c", c=65)[:, :, 64:65], 1.0
                    )
                    ps = psw.tile([128, 256], f32, tag="pw", name=f"psv{tk}")
                    for ch in range(NDC):
                        nc.tensor.matmul(
                            ps[:],
                            xsl(tk // 4, ch, (tk % 4) * 128, (tk % 4) * 128 + 128),
                            wv_t[:, ch * 256 : (ch + 1) * 256],
                            start=(ch == 0),
                            stop=(ch == NDC - 1),
                        )
                    nc.vector.tensor_copy(
                        vt.rearrange("p (h c) -> p h c", c=65)[:, :, 0:64],
                        ps.rearrange("p (h c) -> p h c", c=64),
                    )
                    vts[tk] = vt

                ot0 = otpool.tile([128, T], bf16, tag="o", name="ot0")
                ot1 = otpool.tile([128, T], bf16, tag="o", name="ot1")
                ots = [ot0, ot1]

                # ---- flat attention pipeline over 128 global kc-slots ----
                # slot s -> unit u = s//16 = (pr = u//4, qt qi = u%4), kc = s%16.
                # Emission: prologue S(0..3); then slot k: [S(k+4)] [filler]
                # [PV(k)]; exp(s) is emitted right after S(s). The 4-slot
                # S-lead keeps the ACT exp stream saturated: S(k+4) only
                # ring-waits exp(k+2), never a PV.
                NSLOT = 128
                pts = [None] * NSLOT
                oaccs = {}

                def emit_S(s):
                    u, kc = s // 16, s % 16
                    pr, qi = u // 4, u % 4
                    goff = (s % 2) * 1024
                    for hh in range(2):
                        nc.tensor.matmul(
                            ring[:, goff + hh * 512 : goff + (hh + 1) * 512],
                            kts[pr][hh * 64 : hh * 64 + 64, kc * 128 : (kc + 1) * 128],
                            qts[pr][hh * 64 : hh * 64 + 64, qi * 512 : qi * 512 + 512],
                            start=True,
                            stop=True,
                        )
                    # softmax scale is baked into Wq host-side; plain exp here
                    pt = ppool.tile([128, 1024], bf16, tag="p", name=f"p_{s}")
                    nc.scalar.activation(
                        pt[:], ring[:, goff : goff + 1024], Exp, bias=0.0, scale=1.0
                    )
                    pts[s] = pt

                def emit_PV(s):
                    u, kc = s // 16, s % 16
                    pr, qi = u // 4, u % 4
                    if kc == 0:
                        oaccs[u] = [
                            pso.tile([65, 512], f32, tag="oa", name=f"o_{u}_{hh}")
                            for hh in range(2)
                        ]
                    for hh in range(2):
                        nc.tensor.matmul(
                            oaccs[u][hh][:],
                            vts[kc][:, (2 * pr + hh) * 65 : (2 * pr + hh + 1) * 65],
                            pts[s][:, hh * 512 : (hh + 1) * 512],
                            start=(kc == 0),
                            stop=(kc == NKC - 1),
                        )
                    pts[s] = None

                # raw O eviction: one fast DVE copy per hh frees the oacc
                # PSUM bank ~0.4us after the unit's last PV; the normalize
                # chain then runs entirely in SBUF, off the critical path.
                raws = {}

                def raw_copy(u):
                    raw = rzpool.tile([64, 2048], f32, tag="rz", name=f"raw_{u}")
                    # Z rows land in a separate partition-base-0 tile: the
                    # custom DVE reciprocal requires base-0 operands
                    zr = rzpool.tile([1, 1024], f32, tag="zr", name=f"zr_{u}")
                    for hh in range(2):
                        nc.vector.tensor_copy(
                            raw[:, hh * 512 : (hh + 1) * 512], oaccs[u][hh][0:64, :]
                        )
                        nc.vector.tensor_copy(
                            zr[0:1, hh * 512 : (hh + 1) * 512], oaccs[u][hh][64:65, :]
                        )
                    raws[u] = (raw, zr)

                def normalize(u):
                    pr, qi = u // 4, u % 4
                    qs = qi * 512
                    raw, zr = raws[u]
                    for hh in range(2):
                        hs = slice(hh * 512, (hh + 1) * 512)
                        rz = rbpool.tile([1, 512], f32, tag="rz", name=f"rzz_{u}_{hh}")
                        nc.vector.reciprocal_approx_fast(rz[0:1, :], zr[0:1, hs])
                        rc = rbpool.tile([64, 512], f32, tag="rb", name=f"rc_{u}_{hh}")
                        nc.gpsimd.partition_broadcast(
                            rc[:, :], rz[0:1, :], channels=64
                        )
                        nc.vector.tensor_tensor(
                            out=ots[pr][hh * 64 : hh * 64 + 64, qs : qs + 512],
                            in0=raw[0:64, hs],
                            in1=rc[:, :],
                            op=MUL,
                        )

                # wo sub-unit j of qt qi: t-chunk tt = 4*qi+j (2 matmul pairs
                # + casts); qi<3 packs 2 chunks per [128,2048] ysb + 1 DMA,
                # the tail qt (qi=3) DMAs each chunk separately.
                ysbs = {}

                def wo_sub(qi, j):
                    tt = 4 * qi + j
                    if j % 2 == 0:
                        ysbs[(qi, j // 2)] = ypool.tile(
                            [128, 2048], bf16, tag="y", name=f"y_{qi}_{j // 2}"
                        )
                    ysb = ysbs[(qi, j // 2)]
                    for eh in range(2):
                        # tail qt borrows the (free) oacc pool for every other
                        # accumulator so the cast evictions don't serialize it
                        pool = pso if qi == 3 and eh == 1 else psw
                        tag = "oa" if pool is pso else "pw"
                        yps = pool.tile([128, 512], f32, tag=tag, name=f"yp_{tt}_{eh}")
                        for r in range(2):
                            nc.tensor.matmul(
                                yps[:],
                                ots[r][:, tt * 128 : (tt + 1) * 128],
                                wo_t[:, r * 1024 + eh * 512 : r * 1024 + (eh + 1) * 512],
                                start=(r == 0),
                                stop=(r == 1),
                            )
                        nc.vector.tensor_copy(
                            ysb[:, (j % 2) * 1024 + eh * 512 : (j % 2) * 1024 + (eh + 1) * 512],
                            yps[:],
                        )
                    if qi == 3:
                        eng = nc.sync if j % 2 == 0 else nc.scalar
                        eng.dma_start(
                            out=y_r[:, tt : tt + 1, :],
                            in_=ysb.rearrange("p (b e) -> p b e", e=1024)[
                                :, j % 2 : j % 2 + 1, :
                            ],
                        )
                    elif j % 2 == 1:
                        nc.sync.dma_start(
                            out=y_r[:, 4 * qi + j - 1 : 4 * qi + j + 1, :],
                            in_=ysb.rearrange("p (b e) -> p b e", e=1024),
                        )

                # ---- filler schedule: slot -> list of thunks; fillers are
                # emitted BEFORE the slot's S so late Q/K quarters can feed
                # the S stream just-in-time ----
                fillers = {}

                def add_filler(slot, fn):
                    fillers.setdefault(slot, []).append(fn)

                def qk_part(q, is_k):  # one Q0/K0 proj quarter + its rope
                    def fn():
                        t_ = kts[0] if is_k else qts[0]
                        project(0, is_k, t_, q)
                        rope(t_, "k0" if is_k else "q0", q)

                    return fn

                # quarter q roped before S(4q) at slot 4q-4; split Q/K halves
                # across two slots to limit per-slot PE insertion
                for q, (sq, sk) in ((1, (0, 0)), (2, (2, 3)), (3, (6, 7))):
                    add_filler(sq, qk_part(q, 0))
                    add_filler(sk, qk_part(q, 1))
                vslots = [1, 4, 4, 5, 8, 8, 9, 10, 11, 12, 13, 14]  # V4..V15
                for i, sl in enumerate(vslots):
                    add_filler(sl, (lambda tk: lambda: vproj(tk))(4 + i))
                pj = 0  # pr1 projections + quarter-ropes woven into units 2-3
                for is_k in range(2):
                    for q in range(4):
                        add_filler(
                            32 + 2 * pj,
                            (lambda ik, qq: lambda: project(
                                1, ik, kts[1] if ik else qts[1], qq
                            ))(is_k, q),
                        )
                        add_filler(
                            33 + 2 * pj,
                            (lambda ik, qq: lambda: rope(
                                kts[1] if ik else qts[1], "k1" if ik else "q1", qq
                            ))(is_k, q),
                        )
                        pj += 1
                # wo(qi) woven into unit 5+qi AFTER each slot's S/PV so it
                # never delays the exp stream; wo(3) is the tail
                postfill = {}
                for qi in range(3):
                    for j in range(4):
                        postfill.setdefault(
                            (5 + qi) * 16 + 2 + 3 * j, []
                        ).append((lambda a, b: lambda: wo_sub(a, b))(qi, j))

                # ---- prologue: warm the PE p-state on the (tiny, landed)
                # bias tile while x/weights stream in, then quarter 0 ----
                warm = psw.tile([128, 512], f32, tag="pw", name="warm")
                for w in range(24):
                    nc.tensor.matmul(
                        warm[0:4, 0:4],
                        bia_t.bitcast(bf16)[:, 0:4],
                        bia_t.bitcast(bf16)[:, 0:4],
                        start=True,
                        stop=True,
                    )
                qk_part(0, 0)()
                qk_part(0, 1)()
                for s in range(4):
                    emit_S(s)
                for tk in range(4):
                    vproj(tk)

                # ---- steady state; PV lags S-emission by 5 slots ----
                for k in range(NSLOT + 1):
                    if k > 16 and k % 16 == 1:
                        normalize(k // 16 - 1)
                    for fn in fillers.get(k, ()):
                        fn()
                    if k + 4 < NSLOT:
                        emit_S(k + 4)
                    if k > 0:
                        emit_PV(k - 1)
                    if k > 0 and k % 16 == 0:
                        raw_copy(k // 16 - 1)
                    for fn in postfill.get(k, ()):
                        fn()

                # ---- tail: wo for the last qt ----
                normalize(7)
                for j in range(4):
                    wo_sub(3, j)

            if iters:
                import concourse.mybir as _mb

                with tc.For_i(
                    0,
                    iters,
                    1,
                    hint_engines=(
                        _mb.EngineType.PE,
                        _mb.EngineType.Activation,
                        _mb.EngineType.DVE,
                        _mb.EngineType.SP,
                        _mb.EngineType.Pool,
                    ),
                    staggered_reset=True,
                ) as _iv:
                    body()
            else:
                body()

    nc.compile()
    return nc


def _host_inputs(x, wq_w, wq_b, wk_w, wk_b, wv_w, wv_b, wo_w, wo_b):
    """Build the 8 per-core input maps (all host-side slicing/packing)."""
    f = np.float32
    x = np.asarray(x, f)
    wq_w = np.asarray(wq_w, f)
    wk_w = np.asarray(wk_w, f)
    wv_w = np.asarray(wv_w, f)
    wo_w = np.asarray(wo_w, f)
    wq_b = np.asarray(wq_b, f)
    wk_b = np.asarray(wk_b, f)
    wv_b = np.asarray(wv_b, f)
    wo_b = np.asarray(wo_b, f)

    def chunkpack(a, ncol):  # [1024, ncol] -> [128, 8*ncol] (D-chunk packed)
        return np.ascontiguousarray(
            a.reshape(NDC, 128, ncol).transpose(1, 0, 2).reshape(128, NDC * ncol)
        )

    # RoPE tables in fp32, mirroring the reference formulas; stored bf16.
    pos = np.arange(T, dtype=f)[:, None]
    idx = np.arange(HALF, dtype=f)[None, :]
    inv_freq = (f(1.0) / (f(ROPE_BASE) ** (idx / f(HALF)))).astype(f)
    ang = pos * inv_freq  # [T, 32]
    cosv, sinv = np.cos(ang).astype(f), np.sin(ang).astype(f)
    cos64 = np.repeat(cosv.T, 2, axis=0)  # [64, T]
    sin64 = np.repeat(sinv.T, 2, axis=0)
    sin64[0::2] *= -1  # rows 2j: -sin, rows 2j+1: +sin
    cos128 = np.tile(cos64, (2, 1))
    sin128 = np.tile(sin64, (2, 1))
    # quarter-interleaved: [cos_q | sin_q] per 512-col t-quarter
    tbl = np.ascontiguousarray(
        np.concatenate(
            [
                np.concatenate(
                    [cos128[:, q * 512 : (q + 1) * 512], sin128[:, q * 512 : (q + 1) * 512]],
                    axis=1,
                )
                for q in range(4)
            ],
            axis=1,
        ).astype(BF16)
    )

    perm64 = np.empty(64, np.int64)
    perm64[0::2] = np.arange(32)
    perm64[1::2] = np.arange(32) + 32

    # x[b]^T quarter-packed: quarter q holds all 8 D-chunks for t in
    # [512q, 512(q+1)): [128, 8ch x 512t]
    xqp = []
    for b in range(B):
        xt = x[b].T.reshape(NDC, 128, 4, 512)  # [ch, p, q, t]
        xqp.append(
            np.ascontiguousarray(
                xt.transpose(2, 1, 0, 3).reshape(4, 128, NDC * 512)
                .transpose(1, 0, 2).reshape(128, 4 * NDC * 512)
            ).astype(BF16)
        )

    in_maps = []
    for c in range(N_CORES):
        b, g = c // 4, c % 4
        heads = np.arange(4 * g, 4 * g + 4)
        v_cols = np.concatenate([np.arange(h * 64, (h + 1) * 64) for h in heads])
        # softmax scale baked into Wq/qb so the exp runs with scale=1.0
        wqk_parts, bia_cols = [], []
        for w_, b_, sc in ((wq_w, wq_b, np.float32(SCALE)), (wk_w, wk_b, np.float32(1.0))):
            for pr in range(2):
                prheads = heads[2 * pr : 2 * pr + 2]
                cols = np.concatenate([h * 64 + perm64 for h in prheads])
                wqk_parts.append((pr, chunkpack(w_[:, cols] * sc, 128)))
                bia_cols.append((pr, b_[cols] * sc))
        # layout: wq0 | wk0 | wq1 | wk1  (each [128, 1024])
        order = [0, 2, 1, 3]  # indices into wqk_parts (built q0,q1,k0,k1)
        wqk = np.concatenate([wqk_parts[i][1] for i in order], axis=1).astype(BF16)
        # bias cols: qb0, qb1, kb0, kb1
        bia = np.stack(
            [bia_cols[0][1], bia_cols[1][1], bia_cols[2][1], bia_cols[3][1]], axis=1
        ).astype(f)
        wvp = chunkpack(wv_w[:, v_cols], 256).astype(BF16)
        wop = np.ascontiguousarray(
            wo_w[v_cols, :]
            .reshape(2, 128, D)
            .transpose(1, 0, 2)
            .reshape(128, 2 * D)
            .astype(BF16)
        )
        in_maps.append(
            {
                "xqp": xqp[b],
                "wqk": np.ascontiguousarray(wqk),
                "wvp": np.ascontiguousarray(wvp),
                "tbl": tbl,
                "bia": np.ascontiguousarray(bia),
                "wop": wop,
            }
        )

    beff = (
        wo_b.astype(np.float64) + wv_b.astype(np.float64) @ wo_w.astype(np.float64)
    ).astype(f)
    return in_maps, beff


def kernel(x, wq_w, wq_b, wk_w, wk_b, wv_w, wv_b, wo_w, wo_b):
    from concourse import bass2jax

    in_maps, beff = _host_inputs(
        x, wq_w, wq_b, wk_w, wk_b, wv_w, wv_b, wo_w, wo_b
    )
    if "nc" not in _ctx:
        _ctx["nc"] = _build_nc(0)
    res = bass2jax.run_bass_via_pjrt(_ctx["nc"], in_maps, n_cores=N_CORES)
    y = np.empty((B, T, D), np.float32)
    for b in range(B):
        acc = np.asarray(res[4 * b]["y"], np.float32)
        for g in range(1, 4):
            acc += np.asarray(res[4 * b + g]["y"], np.float32)
        y[b] = acc + beff[None, :]
    return y


# revision 54
# speedup vs baseline: 1.1693x; 1.0151x over previous
# Multi-head self-attention with RoPE on 8 Trainium2 NeuronCores.
#
# Sharding: batch x head-group. Core c handles batch b = c//4 and heads
# 4*(c%4) .. 4*(c%4)+3 (4 of 16 heads), organized as 2 "pr" pairs of 2
# heads. Each core computes Q/K/V projections for its heads from the full
# (transposed) x[b], runs attention, and produces a partial output
# projection Y_partial = O_core^T.T @ Wo[rows-of-its-heads] in bf16. The
# host sums the four partials per batch (in f32) and adds the bias terms.
#
# Everything on-device is bf16 (inputs, Q/K/V, P=exp(S), O, Wo, Y) with
# f32 PSUM accumulation; measured end-to-end rel err ~7e-3 vs the f32
# reference (budget 2e-2). The softmax scale (1/8) is baked into Wq/qb
# host-side so the exp runs with scale=1.0.
#
# The kernel is ACT(exp)-throughput-bound: the Scalar/ACT engine streams
# one [128,1024] exp per kc-slot at ~1.11us + ~0.55us fixed instruction
# overhead (measured free-run cadence 1.66us), 128 slots total. The whole
# schedule is built around keeping that stream saturated:
#
#   - flat 128-slot pipeline over units (pr, qt): at slot k the S-pair
#     for slot k+4 is emitted (it only ring-WARs on exp(k+2), never on a
#     PV), then fillers, then PV(k-1). PSUM: exp ring [128,2048] (2
#     groups) + 2 oacc banks + 2 proj/V/WO banks.
#   - V tiles 4..15 and the pr1 projections+ropes are woven in as
#     pre-S fillers (they produce data the S stream needs); the output
#     projection (WO) is woven in AFTER each slot's S/PV so its matmuls
#     never delay the exp stream.
#   - at each unit boundary the [65,512] O accumulators (row 64 = Z via
#     the ones-column-in-V trick) are evicted raw to SBUF with two fast
#     DVE copies; the softmax normalization (reciprocal_approx_fast +
#     gpsimd partition_broadcast + DVE multiply) runs later, off the
#     critical path. NOTE: custom DVE ops (reciprocal_approx_*) need
#     SBUF operands at partition base 0 - PSUM reads or partition-offset
#     inputs silently corrupt on HW.
#   - input DMAs are staged across both HWDGE rings so the first
#     projection matmul starts ~15us in (x quarter 0 split in halves,
#     first-needed weights first, RoPE tables quarter-interleaved).
#
# RoPE: head-dim rows are pair-interleaved (d' = [0,32,1,33,...]) via a
# host permutation of Wq/Wk columns so the rotate-half partner lives on
# the adjacent partition; a DVE stream_shuffle (pair swap on u32-bitcast
# bf16 pairs) + 2 muls + 1 add apply the rotation per 512-col t-quarter.

import os
import sys

import numpy as np

try:
    import ml_dtypes

    BF16 = np.dtype(ml_dtypes.bfloat16)
except ImportError:  # pragma: no cover
    BF16 = None

for _p in ("/opt/trn_rl_repo", os.path.expanduser("~/.axon_site/_ro/trn_rl_repo")):
    if os.path.isdir(_p) and _p not in sys.path:
        sys.path.insert(0, _p)

B, T, D = 2, 2048, 1024
NHEADS, HD, HALF = 16, 64, 32
HPC = 4  # heads per core
N_CORES = 8
ROPE_BASE = 10000.0
SCALE = float(HD) ** -0.5  # 0.125
NDC = D // 128  # 8 contraction chunks for the projections
NKC = T // 128  # 16 k chunks per head

_SHUF_MASK = [i ^ 1 for i in range(32)]

_ctx: dict = {}


def _build_nc(iters: int = 0, phase: str = "full"):
    import concourse.bacc as bacc
    import concourse.mybir as mybir
    import concourse.tile as tile

    f32 = mybir.dt.float32
    bf16 = mybir.dt.bfloat16
    u32 = mybir.dt.uint32
    i32 = mybir.dt.int32
    Exp = mybir.ActivationFunctionType.Exp
    MUL = mybir.AluOpType.mult

    nc = bacc.Bacc("TRN2", target_bir_lowering=False, debug=False)

    # packed inputs (see _host_inputs for layouts)
    xq_d = nc.dram_tensor("xqp", [128, 4 * NDC * 512], bf16, kind="ExternalInput").ap()
    wqk_d = nc.dram_tensor("wqk", [128, 4096], bf16, kind="ExternalInput").ap()
    wvp_d = nc.dram_tensor("wvp", [128, 2048], bf16, kind="ExternalInput").ap()
    tbl_d = nc.dram_tensor("tbl", [128, 4096], bf16, kind="ExternalInput").ap()
    bia_d = nc.dram_tensor("bia", [128, 4], f32, kind="ExternalInput").ap()
    wo_d = nc.dram_tensor("wop", [128, 2048], bf16, kind="ExternalInput").ap()
    y_d = nc.dram_tensor("y", [T, D], bf16, kind="ExternalOutput").ap()
    y_r = y_d.rearrange("(b p) e -> p b e", p=128)  # [128, 16, 1024]

    with tile.TileContext(nc) as tc:
        with (
            tc.tile_pool(name="xpool", bufs=4) as xpool,
            tc.tile_pool(name="wpool", bufs=1) as wpool,
            tc.tile_pool(name="qkpool", bufs=4) as qkpool,
            tc.tile_pool(name="shpool", bufs=2) as shpool,
            tc.tile_pool(name="vpool", bufs=16) as vpool,
            tc.tile_pool(name="ppool", bufs=8) as ppool,
            tc.tile_pool(name="otpool", bufs=2) as otpool,
            tc.tile_pool(name="ypool", bufs=2) as ypool,
            tc.tile_pool(name="rzpool", bufs=4) as rzpool,
            tc.tile_pool(name="rbpool", bufs=4) as rbpool,
            tc.tile_pool(name="pring", bufs=1, space="PSUM") as pring,
            tc.tile_pool(name="pso", bufs=2, space="PSUM") as pso,
            tc.tile_pool(name="psw", bufs=2, space="PSUM") as psw,
        ):

            def body():
                # ---- staged input DMAs across both HWDGE rings ----
                # sync:   bia, wq0, wk0, xq0b, wvp, xq2, wqk[2048:], wo, y out
                # scalar: xq0a, tbl, xq1, xq3, y out (tail)
                bia_t = wpool.tile([128, 4], f32, tag="bia", name="bia_t")
                nc.sync.dma_start(out=bia_t[:], in_=bia_d)
                wqk_t = wpool.tile([128, 4096], bf16, tag="wqk", name="wqk_t")
                nc.sync.dma_start(out=wqk_t[:, 0:2048], in_=wqk_d[:, 0:2048])
                # x quarter 0 split into chunk-halves across both rings
                xq0a = xpool.tile([128, 2048], bf16, tag="xh", name="xq0a")
                nc.scalar.dma_start(out=xq0a[:], in_=xq_d[:, 0:2048])
                xq0b = xpool.tile([128, 2048], bf16, tag="xh", name="xq0b")
                nc.sync.dma_start(out=xq0b[:], in_=xq_d[:, 2048:4096])
                xqt = [None]
                for q in range(1, 4):
                    xq = xpool.tile([128, NDC * 512], bf16, tag="x", name=f"xq{q}")
                    xqt.append(xq)
                nc.scalar.dma_start(out=xqt[1][:], in_=xq_d[:, 4096:8192])
                tbl_t = wpool.tile([128, 4096], bf16, tag="tbl", name="tbl_t")
                nc.sync.dma_start(out=tbl_t[:, 0:2048], in_=tbl_d[:, 0:2048])
                wv_t = wpool.tile([128, 2048], bf16, tag="wv", name="wv_t")
                nc.sync.dma_start(out=wv_t[:], in_=wvp_d)
                nc.scalar.dma_start(out=tbl_t[:, 2048:4096], in_=tbl_d[:, 2048:4096])
                nc.sync.dma_start(out=xqt[2][:], in_=xq_d[:, 8192:12288])
                nc.scalar.dma_start(out=xqt[3][:], in_=xq_d[:, 12288:16384])
                nc.sync.dma_start(out=wqk_t[:, 2048:4096], in_=wqk_d[:, 2048:4096])
                wo_t = wpool.tile([128, 2048], bf16, tag="wo", name="wo_t")
                nc.sync.dma_start(out=wo_t[:], in_=wo_d)

                def xsl(q, ch, lo, hi):  # x slice for quarter q, chunk ch
                    if q == 0:
                        t_ = xq0a if ch < 4 else xq0b
                        return t_[:, (ch % 4) * 512 + lo : (ch % 4) * 512 + hi]
                    return xqt[q][:, ch * 512 + lo : ch * 512 + hi]

                ring = pring.tile([128, 2048], f32, tag="ring", name="ring")

                # ---- Q/K projections; eviction = DVE tensor_scalar (+bias) ----
                def project(pr, is_k, dst, q):
                    base = pr * 2048 + (1024 if is_k else 0)
                    ps = psw.tile([128, 512], f32, tag="pw", name=f"ps_{pr}{is_k}{q}")
                    for ch in range(NDC):
                        nc.tensor.matmul(
                            ps[:],
                            wqk_t[:, base + ch * 128 : base + (ch + 1) * 128],
                            xsl(q, ch, 0, 512),
                            start=(ch == 0),
                            stop=(ch == NDC - 1),
                        )
                    nc.vector.tensor_scalar_add(
                        dst[:, q * 512 : (q + 1) * 512],
                        ps[:],
                        bia_t[:, is_k * 2 + pr : is_k * 2 + pr + 1],
                    )

                def rope(t_, name, q):  # rope one 512-col t-quarter in place
                    # tbl layout is quarter-interleaved: [cos_q | sin_q] x 4
                    sl = slice(q * 512, (q + 1) * 512)
                    cos_q = tbl_t[:, q * 1024 : q * 1024 + 512]
                    sin_q = tbl_t[:, q * 1024 + 512 : (q + 1) * 1024]
                    sh = shpool.tile([128, 512], bf16, tag="sh", name=f"sh_{name}{q}")
                    nc.vector.stream_shuffle(
                        sh.bitcast(u32)[:], t_.bitcast(u32)[:, q * 256 : (q + 1) * 256],
                        _SHUF_MASK,
                    )
                    nc.vector.tensor_tensor(
                        out=t_[:, sl], in0=t_[:, sl], in1=cos_q, op=MUL
                    )
                    nc.vector.tensor_tensor(out=sh[:], in0=sh[:], in1=sin_q, op=MUL)
                    nc.vector.tensor_tensor(
                        out=t_[:, sl], in0=t_[:, sl], in1=sh[:], op=mybir.AluOpType.add
                    )

                qts, kts = [], []
                for pr in range(2):
                    qts.append(qkpool.tile([128, T], bf16, tag="qk", name=f"qt{pr}"))
                    kts.append(qkpool.tile([128, T], bf16, tag="qk", name=f"kt{pr}"))

                # ---- V projection (bf16 tiles, ones col via memset) ----
                vts = [None] * NKC

                def vproj(tk):
                    vt = vpool.tile([128, HPC * 65], bf16, tag="v", name=f"v{tk}")
                    nc.vector.memset(
                        vt.rearrange("p (h c) -> p h c", c=65)[:, :, 64:65], 1.0
                    )
                    ps = psw.tile([128, 256], f32, tag="pw", name=f"psv{tk}")
                    for ch in range(NDC):
                        nc.tensor.matmul(
                            ps[:],
                            xsl(tk // 4, ch, (tk % 4) * 128, (tk % 4) * 128 + 128),
                            wv_t[:, ch * 256 : (ch + 1) * 256],
                            start=(ch == 0),
                            stop=(ch == NDC - 1),
                        )
                    nc.vector.tensor_copy(
                        vt.rearrange("p (h c) -> p h c", c=65)[:, :, 0:64],
                        ps.rearrange("p (h c) -> p h c", c=64),
                    )
                    vts[tk] = vt

                ot0 = otpool.tile([128, T], bf16, tag="o", name="ot0")
                ot1 = otpool.tile([128, T], bf16, tag="o", name="ot1")
                ots = [ot0, ot1]

                # ---- flat attention pipeline over 128 global kc-slots ----
                # slot s -> unit u = s//16 = (pr = u//4, qt qi = u%4), kc = s%16.
                # Emission: prologue S(0..3); then slot k: [S(k+4)] [filler]
                # [PV(k)]; exp(s) is emitted right after S(s). The 4-slot
                # S-lead keeps the ACT exp stream saturated: S(k+4) only
                # ring-waits exp(k+2), never a PV.
                NSLOT = 128
                pts = [None] * NSLOT
                oaccs = {}

                def emit_S(s):
                    u, kc = s // 16, s % 16
                    pr, qi = u // 4, u % 4
                    goff = (s % 2) * 1024
                    for hh in range(2):
                        nc.tensor.matmul(
                            ring[:, goff + hh * 512 : goff + (hh + 1) * 512],
                            kts[pr][hh * 64 : hh * 64 + 64, kc * 128 : (kc + 1) * 128],
                            qts[pr][hh * 64 : hh * 64 + 64, qi * 512 : qi * 512 + 512],
                            start=True,
                            stop=True,
                        )
                    # softmax scale is baked into Wq host-side; plain exp here
                    pt = ppool.tile([128, 1024], bf16, tag="p", name=f"p_{s}")
                    nc.scalar.activation(
                        pt[:], ring[:, goff : goff + 1024], Exp, bias=0.0, scale=1.0
                    )
                    pts[s] = pt

                def emit_PV(s):
                    u, kc = s // 16, s % 16
                    pr, qi = u // 4, u % 4
                    if kc == 0:
                        oaccs[u] = [
                            pso.tile([65, 512], f32, tag="oa", name=f"o_{u}_{hh}")
                            for hh in range(2)
                        ]
                    for hh in range(2):
                        nc.tensor.matmul(
                            oaccs[u][hh][:],
                            vts[kc][:, (2 * pr + hh) * 65 : (2 * pr + hh + 1) * 65],
                            pts[s][:, hh * 512 : (hh + 1) * 512],
                            start=(kc == 0),
                            stop=(kc == NKC - 1),
                        )
                    pts[s] = None

                # raw O eviction: one fast DVE copy per hh frees the oacc
                # PSUM bank ~0.4us after the unit's last PV; the normalize
                # chain then runs entirely in SBUF, off the critical path.
                raws = {}

                def raw_copy(u):
                    raw = rzpool.tile([64, 2048], f32, tag="rz", name=f"raw_{u}")
                    # Z rows land in a separate partition-base-0 tile: the
                    # custom DVE reciprocal requires base-0 operands
                    zr = rzpool.tile([1, 1024], f32, tag="zr", name=f"zr_{u}")
                    for hh in range(2):
                        nc.vector.tensor_copy(
                            raw[:, hh * 512 : (hh + 1) * 512], oaccs[u][hh][0:64, :]
                        )
                        nc.vector.tensor_copy(
                            zr[0:1, hh * 512 : (hh + 1) * 512], oaccs[u][hh][64:65, :]
                        )
                    raws[u] = (raw, zr)

                def normalize(u):
                    pr, qi = u // 4, u % 4
                    qs = qi * 512
                    raw, zr = raws[u]
                    for hh in range(2):
                        hs = slice(hh * 512, (hh + 1) * 512)
                        rz = rbpool.tile([1, 512], f32, tag="rz", name=f"rzz_{u}_{hh}")
                        nc.vector.reciprocal_approx_fast(rz[0:1, :], zr[0:1, hs])
                        rc = rbpool.tile([64, 512], f32, tag="rb", name=f"rc_{u}_{hh}")
                        nc.gpsimd.partition_broadcast(
                            rc[:, :], rz[0:1, :], channels=64
                        )
                        nc.vector.tensor_tensor(
                            out=ots[pr][hh * 64 : hh * 64 + 64, qs : qs + 512],
                            in0=raw[0:64, hs],
                            in1=rc[:, :],
                            op=MUL,
                        )

                # wo sub-unit j of qt qi: t-chunk tt = 4*qi+j (2 matmul pairs
                # + casts); qi<3 packs 2 chunks per [128,2048] ysb + 1 DMA,
                # the tail qt (qi=3) DMAs each chunk separately.
                ysbs = {}

                def wo_sub(qi, j):
                    tt = 4 * qi + j
                    if j % 2 == 0:
                        ysbs[(qi, j // 2)] = ypool.tile(
                            [128, 2048], bf16, tag="y", name=f"y_{qi}_{j // 2}"
                        )
                    ysb = ysbs[(qi, j // 2)]
                    for eh in range(2):
                        # tail qt borrows the (free) oacc pool for every other
                        # accumulator so the cast evictions don't serialize it
                        pool = pso if qi == 3 and eh == 1 else psw
                        tag = "oa" if pool is pso else "pw"
                        yps = pool.tile([128, 512], f32, tag=tag, name=f"yp_{tt}_{eh}")
                        for r in range(2):
                            nc.tensor.matmul(
                                yps[:],
                                ots[r][:, tt * 128 : (tt + 1) * 128],
                                wo_t[:, r * 1024 + eh * 512 : r * 1024 + (eh + 1) * 512],
                                start=(r == 0),
                                stop=(r == 1),
                            )
                        nc.vector.tensor_copy(
                            ysb[:, (j % 2) * 1024 + eh * 512 : (j % 2) * 1024 + (eh + 1) * 512],
                            yps[:],
                        )
                    if qi == 3:
                        eng = nc.sync if j % 2 == 0 else nc.scalar
                        eng.dma_start(
                            out=y_r[:, tt : tt + 1, :],
                            in_=ysb.rearrange("p (b e) -> p b e", e=1024)[
                                :, j % 2 : j % 2 + 1, :
                            ],
                        )
                    elif j % 2 == 1:
                        nc.sync.dma_start(
                            out=y_r[:, 4 * qi + j - 1 : 4 * qi + j + 1, :],
                            in_=ysb.rearrange("p (b e) -> p b e", e=1024),
                        )

                # ---- filler schedule: slot -> list of thunks; fillers are
                # emitted BEFORE the slot's S so late Q/K quarters can feed
                # the S stream just-in-time ----
                fillers = {}

                def add_filler(slot, fn):
                    fillers.setdefault(slot, []).append(fn)

                def qk_part(q, is_k):  # one Q0/K0 proj quarter + its rope
                    def fn():
                        t_ = kts[0] if is_k else qts[0]
                        project(0, is_k, t_, q)
                        rope(t_, "k0" if is_k else "q0", q)

                    return fn

                # quarter q roped before S(4q) at slot 4q-4; split Q/K halves
                # across two slots to limit per-slot PE insertion
                for q, (sq, sk) in ((1, (0, 0)), (2, (2, 3)), (3, (6, 7))):
                    add_filler(sq, qk_part(q, 0))
                    add_filler(sk, qk_part(q, 1))
                vslots = [1, 4, 4, 5, 8, 8, 9, 10, 11, 12, 13, 14]  # V4..V15
                for i, sl in enumerate(vslots):
                    add_filler(sl, (lambda tk: lambda: vproj(tk))(4 + i))
                pj = 0  # pr1 projections + quarter-ropes woven into units 2-3
                for is_k in range(2):
                    for q in range(4):
                        add_filler(
                            32 + 2 * pj,
                            (lambda ik, qq: lambda: project(
                                1, ik, kts[1] if ik else qts[1], qq
                            ))(is_k, q),
                        )
                        add_filler(
                            33 + 2 * pj,
                            (lambda ik, qq: lambda: rope(
                                kts[1] if ik else qts[1], "k1" if ik else "q1", qq
                            ))(is_k, q),
                        )
                        pj += 1
                # wo(qi) woven into unit 5+qi AFTER each slot's S/PV so it
                # never delays the exp stream; wo(3) is the tail
                postfill = {}
                for qi in range(3):
                    for j in range(4):
                        postfill.setdefault(
                            (5 + qi) * 16 + 2 + 3 * j, []
                        ).append((lambda a, b: lambda: wo_sub(a, b))(qi, j))

                # ---- prologue: warm the PE p-state on the (tiny, landed)
                # bias tile while x/weights stream in, then quarter 0 ----
                warm = psw.tile([128, 512], f32, tag="pw", name="warm")
                for w in range(24):
                    nc.tensor.matmul(
                        warm[0:4, 0:4],
                        bia_t.bitcast(bf16)[:, 0:4],
                        bia_t.bitcast(bf16)[:, 0:4],
                        start=True,
                        stop=True,
                    )
                qk_part(0, 0)()
                qk_part(0, 1)()
                for s in range(4):
                    emit_S(s)
                for tk in range(4):
                    vproj(tk)

                # ---- steady state; PV lags S-emission by 5 slots ----
                for k in range(NSLOT + 1):
                    if k > 16 and k % 16 == 1:
                        normalize(k // 16 - 1)
                    for fn in fillers.get(k, ()):
                        fn()
                    if k + 4 < NSLOT:
                        emit_S(k + 4)
                    if k > 0:
                        emit_PV(k - 1)
                    if k > 0 and k % 16 == 0 and k // 16 - 1 != 7:
                        raw_copy(k // 16 - 1)
                    for fn in postfill.get(k, ()):
                        fn()

                # ---- tail: wo for the last qt; its normalize reads the
                # oacc PSUM directly (skips the raw-copy hop) ----
                for hh in range(2):
                    rz = rzpool.tile([1, 1024], f32, tag="zr", name=f"rzt_{hh}")
                    nc.vector.tensor_copy(rz[0:1, 0:512], oaccs[7][hh][64:65, :])
                    nc.vector.reciprocal_approx_fast(
                        rz[0:1, 512:1024], rz[0:1, 0:512]
                    )
                    rb = rbpool.tile([64, 512], f32, tag="rb", name=f"rbt_{hh}")
                    nc.gpsimd.partition_broadcast(
                        rb[:, :], rz[0:1, 512:1024], channels=64
                    )
                    nc.vector.tensor_tensor(
                        out=ots[1][hh * 64 : hh * 64 + 64, 3 * 512 : 4 * 512],
                        in0=oaccs[7][hh][0:64, :],
                        in1=rb[:, :],
                        op=MUL,
                    )
                for j in range(4):
                    wo_sub(3, j)

            if iters:
                import concourse.mybir as _mb

                with tc.For_i(
                    0,
                    iters,
                    1,
                    hint_engines=(
                        _mb.EngineType.PE,
                        _mb.EngineType.Activation,
                        _mb.EngineType.DVE,
                        _mb.EngineType.SP,
                        _mb.EngineType.Pool,
                    ),
                    staggered_reset=True,
                ) as _iv:
                    body()
            else:
                body()

    nc.compile()
    return nc


def _host_inputs(x, wq_w, wq_b, wk_w, wk_b, wv_w, wv_b, wo_w, wo_b):
    """Build the 8 per-core input maps (all host-side slicing/packing)."""
    f = np.float32
    x = np.asarray(x, f)
    wq_w = np.asarray(wq_w, f)
    wk_w = np.asarray(wk_w, f)
    wv_w = np.asarray(wv_w, f)
    wo_w = np.asarray(wo_w, f)
    wq_b = np.asarray(wq_b, f)
    wk_b = np.asarray(wk_b, f)
    wv_b = np.asarray(wv_b, f)
    wo_b = np.asarray(wo_b, f)

    def chunkpack(a, ncol):  # [1024, ncol] -> [128, 8*ncol] (D-chunk packed)
        return np.ascontiguousarray(
            a.reshape(NDC, 128, ncol).transpose(1, 0, 2).reshape(128, NDC * ncol)
        )

    # RoPE tables in fp32, mirroring the reference formulas; stored bf16.
    pos = np.arange(T, dtype=f)[:, None]
    idx = np.arange(HALF, dtype=f)[None, :]
    inv_freq = (f(1.0) / (f(ROPE_BASE) ** (idx / f(HALF)))).astype(f)
    ang = pos * inv_freq  # [T, 32]
    cosv, sinv = np.cos(ang).astype(f), np.sin(ang).astype(f)
    cos64 = np.repeat(cosv.T, 2, axis=0)  # [64, T]
    sin64 = np.repeat(sinv.T, 2, axis=0)
    sin64[0::2] *= -1  # rows 2j: -sin, rows 2j+1: +sin
    cos128 = np.tile(cos64, (2, 1))
    sin128 = np.tile(sin64, (2, 1))
    # quarter-interleaved: [cos_q | sin_q] per 512-col t-quarter
    tbl = np.ascontiguousarray(
        np.concatenate(
            [
                np.concatenate(
                    [cos128[:, q * 512 : (q + 1) * 512], sin128[:, q * 512 : (q + 1) * 512]],
                    axis=1,
                )
                for q in range(4)
            ],
            axis=1,
        ).astype(BF16)
    )

    perm64 = np.empty(64, np.int64)
    perm64[0::2] = np.arange(32)
    perm64[1::2] = np.arange(32) + 32

    # x[b]^T quarter-packed: quarter q holds all 8 D-chunks for t in
    # [512q, 512(q+1)): [128, 8ch x 512t]
    xqp = []
    for b in range(B):
        xt = x[b].T.reshape(NDC, 128, 4, 512)  # [ch, p, q, t]
        xqp.append(
            np.ascontiguousarray(
                xt.transpose(2, 1, 0, 3).reshape(4, 128, NDC * 512)
                .transpose(1, 0, 2).reshape(128, 4 * NDC * 512)
            ).astype(BF16)
        )

    in_maps = []
    for c in range(N_CORES):
        b, g = c // 4, c % 4
        heads = np.arange(4 * g, 4 * g + 4)
        v_cols = np.concatenate([np.arange(h * 64, (h + 1) * 64) for h in heads])
        # softmax scale baked into Wq/qb so the exp runs with scale=1.0
        wqk_parts, bia_cols = [], []
        for w_, b_, sc in ((wq_w, wq_b, np.float32(SCALE)), (wk_w, wk_b, np.float32(1.0))):
            for pr in range(2):
                prheads = heads[2 * pr : 2 * pr + 2]
                cols = np.concatenate([h * 64 + perm64 for h in prheads])
                wqk_parts.append((pr, chunkpack(w_[:, cols] * sc, 128)))
                bia_cols.append((pr, b_[cols] * sc))
        # layout: wq0 | wk0 | wq1 | wk1  (each [128, 1024])
        order = [0, 2, 1, 3]  # indices into wqk_parts (built q0,q1,k0,k1)
        wqk = np.concatenate([wqk_parts[i][1] for i in order], axis=1).astype(BF16)
        # bias cols: qb0, qb1, kb0, kb1
        bia = np.stack(
            [bia_cols[0][1], bia_cols[1][1], bia_cols[2][1], bia_cols[3][1]], axis=1
        ).astype(f)
        wvp = chunkpack(wv_w[:, v_cols], 256).astype(BF16)
        wop = np.ascontiguousarray(
            wo_w[v_cols, :]
            .reshape(2, 128, D)
            .transpose(1, 0, 2)
            .reshape(128, 2 * D)
            .astype(BF16)
        )
        in_maps.append(
            {
                "xqp": xqp[b],
                "wqk": np.ascontiguousarray(wqk),
                "wvp": np.ascontiguousarray(wvp),
                "tbl": tbl,
                "bia": np.ascontiguousarray(bia),
                "wop": wop,
            }
        )

    beff = (
        wo_b.astype(np.float64) + wv_b.astype(np.float64) @ wo_w.astype(np.float64)
    ).astype(f)
    return in_maps, beff


def kernel(x, wq_w, wq_b, wk_w, wk_b, wv_w, wv_b, wo_w, wo_b):
    from concourse import bass2jax

    in_maps, beff = _host_inputs(
        x, wq_w, wq_b, wk_w, wk_b, wv_w, wv_b, wo_w, wo_b
    )
    if "nc" not in _ctx:
        _ctx["nc"] = _build_nc(0)
    res = bass2jax.run_bass_via_pjrt(_ctx["nc"], in_maps, n_cores=N_CORES)
    y = np.empty((B, T, D), np.float32)
    for b in range(B):
        acc = np.asarray(res[4 * b]["y"], np.float32)
        for g in range(1, 4):
            acc += np.asarray(res[4 * b + g]["y"], np.float32)
        y[b] = acc + beff[None, :]
    return y


# revision 56
# speedup vs baseline: 1.1704x; 1.0010x over previous
# Multi-head self-attention with RoPE on 8 Trainium2 NeuronCores.
#
# Sharding: batch x head-group. Core c handles batch b = c//4 and heads
# 4*(c%4) .. 4*(c%4)+3 (4 of 16 heads), organized as 2 "pr" pairs of 2
# heads. Each core computes Q/K/V projections for its heads from the full
# (transposed) x[b], runs attention, and produces a partial output
# projection Y_partial = O_core^T.T @ Wo[rows-of-its-heads] in bf16. The
# host sums the four partials per batch (in f32) and adds the bias terms.
#
# Everything on-device is bf16 (inputs, Q/K/V, P=exp(S), O, Wo, Y) with
# f32 PSUM accumulation; measured end-to-end rel err ~7e-3 vs the f32
# reference (budget 2e-2). The softmax scale (1/8) is baked into Wq/qb
# host-side so the exp runs with scale=1.0.
#
# The kernel is ACT(exp)-throughput-bound: the Scalar/ACT engine streams
# one [128,1024] exp per kc-slot at ~1.11us + ~0.55us fixed instruction
# overhead (measured free-run cadence 1.66us), 128 slots total. The whole
# schedule is built around keeping that stream saturated:
#
#   - flat 128-slot pipeline over units (pr, qt): at slot k the S-pair
#     for slot k+4 is emitted (it only ring-WARs on exp(k+2), never on a
#     PV), then fillers, then PV(k-1). PSUM: exp ring [128,2048] (2
#     groups) + 2 oacc banks + 2 proj/V/WO banks.
#   - V tiles 4..15 and the pr1 projections+ropes are woven in as
#     pre-S fillers (they produce data the S stream needs); the output
#     projection (WO) is woven in AFTER each slot's S/PV so its matmuls
#     never delay the exp stream.
#   - at each unit boundary the [65,512] O accumulators (row 64 = Z via
#     the ones-column-in-V trick) are evicted raw to SBUF with two fast
#     DVE copies; the softmax normalization (reciprocal_approx_fast +
#     gpsimd partition_broadcast + DVE multiply) runs later, off the
#     critical path. NOTE: custom DVE ops (reciprocal_approx_*) need
#     SBUF operands at partition base 0 - PSUM reads or partition-offset
#     inputs silently corrupt on HW.
#   - input DMAs are staged across both HWDGE rings so the first
#     projection matmul starts ~15us in (x quarter 0 split in halves,
#     first-needed weights first, RoPE tables quarter-interleaved).
#
# RoPE: head-dim rows are pair-interleaved (d' = [0,32,1,33,...]) via a
# host permutation of Wq/Wk columns so the rotate-half partner lives on
# the adjacent partition; a DVE stream_shuffle (pair swap on u32-bitcast
# bf16 pairs) + 2 muls + 1 add apply the rotation per 512-col t-quarter.

import os
import sys

import numpy as np

try:
    import ml_dtypes

    BF16 = np.dtype(ml_dtypes.bfloat16)
except ImportError:  # pragma: no cover
    BF16 = None

for _p in ("/opt/trn_rl_repo", os.path.expanduser("~/.axon_site/_ro/trn_rl_repo")):
    if os.path.isdir(_p) and _p not in sys.path:
        sys.path.insert(0, _p)

B, T, D = 2, 2048, 1024
NHEADS, HD, HALF = 16, 64, 32
HPC = 4  # heads per core
N_CORES = 8
ROPE_BASE = 10000.0
SCALE = float(HD) ** -0.5  # 0.125
NDC = D // 128  # 8 contraction chunks for the projections
NKC = T // 128  # 16 k chunks per head

_SHUF_MASK = [i ^ 1 for i in range(32)]

_ctx: dict = {}


def _build_nc(iters: int = 0, phase: str = "full"):
    import concourse.bacc as bacc
    import concourse.mybir as mybir
    import concourse.tile as tile

    f32 = mybir.dt.float32
    bf16 = mybir.dt.bfloat16
    u32 = mybir.dt.uint32
    i32 = mybir.dt.int32
    Exp = mybir.ActivationFunctionType.Exp
    MUL = mybir.AluOpType.mult

    nc = bacc.Bacc("TRN2", target_bir_lowering=False, debug=False)

    # packed inputs (see _host_inputs for layouts)
    xq_d = nc.dram_tensor("xqp", [128, 4 * NDC * 512], bf16, kind="ExternalInput").ap()
    wqk_d = nc.dram_tensor("wqk", [128, 4096], bf16, kind="ExternalInput").ap()
    wvp_d = nc.dram_tensor("wvp", [128, 2048], bf16, kind="ExternalInput").ap()
    tbl_d = nc.dram_tensor("tbl", [128, 4096], bf16, kind="ExternalInput").ap()
    bia_d = nc.dram_tensor("bia", [128, 4], f32, kind="ExternalInput").ap()
    wo_d = nc.dram_tensor("wop", [128, 2048], bf16, kind="ExternalInput").ap()
    y_d = nc.dram_tensor("y", [T, D], bf16, kind="ExternalOutput").ap()
    y_r = y_d.rearrange("(b p) e -> p b e", p=128)  # [128, 16, 1024]

    with tile.TileContext(nc) as tc:
        with (
            tc.tile_pool(name="xpool", bufs=4) as xpool,
            tc.tile_pool(name="wpool", bufs=1) as wpool,
            tc.tile_pool(name="qkpool", bufs=4) as qkpool,
            tc.tile_pool(name="shpool", bufs=2) as shpool,
            tc.tile_pool(name="vpool", bufs=16) as vpool,
            tc.tile_pool(name="ppool", bufs=8) as ppool,
            tc.tile_pool(name="otpool", bufs=2) as otpool,
            tc.tile_pool(name="ypool", bufs=2) as ypool,
            tc.tile_pool(name="rzpool", bufs=4) as rzpool,
            tc.tile_pool(name="rbpool", bufs=4) as rbpool,
            tc.tile_pool(name="pring", bufs=1, space="PSUM") as pring,
            tc.tile_pool(name="pso", bufs=2, space="PSUM") as pso,
            tc.tile_pool(name="psw", bufs=2, space="PSUM") as psw,
        ):

            def body():
                # ---- staged input DMAs across both HWDGE rings ----
                # sync:   bia, wq0, wk0, xq0b, wvp, xq2, wqk[2048:], wo, y out
                # scalar: xq0a, tbl, xq1, xq3, y out (tail)
                bia_t = wpool.tile([128, 4], f32, tag="bia", name="bia_t")
                nc.sync.dma_start(out=bia_t[:], in_=bia_d)
                wqk_t = wpool.tile([128, 4096], bf16, tag="wqk", name="wqk_t")
                nc.sync.dma_start(out=wqk_t[:, 0:2048], in_=wqk_d[:, 0:2048])
                # x quarter 0 split into chunk-halves across both rings
                xq0a = xpool.tile([128, 2048], bf16, tag="xh", name="xq0a")
                nc.scalar.dma_start(out=xq0a[:], in_=xq_d[:, 0:2048])
                xq0b = xpool.tile([128, 2048], bf16, tag="xh", name="xq0b")
                nc.sync.dma_start(out=xq0b[:], in_=xq_d[:, 2048:4096])
                xqt = [None]
                for q in range(1, 4):
                    xq = xpool.tile([128, NDC * 512], bf16, tag="x", name=f"xq{q}")
                    xqt.append(xq)
                nc.scalar.dma_start(out=xqt[1][:], in_=xq_d[:, 4096:8192])
                tbl_t = wpool.tile([128, 4096], bf16, tag="tbl", name="tbl_t")
                nc.sync.dma_start(out=tbl_t[:, 0:2048], in_=tbl_d[:, 0:2048])
                wv_t = wpool.tile([128, 2048], bf16, tag="wv", name="wv_t")
                nc.sync.dma_start(out=wv_t[:], in_=wvp_d)
                nc.scalar.dma_start(out=tbl_t[:, 2048:4096], in_=tbl_d[:, 2048:4096])
                nc.sync.dma_start(out=xqt[2][:], in_=xq_d[:, 8192:12288])
                nc.scalar.dma_start(out=xqt[3][:], in_=xq_d[:, 12288:16384])
                nc.sync.dma_start(out=wqk_t[:, 2048:4096], in_=wqk_d[:, 2048:4096])
                wo_t = wpool.tile([128, 2048], bf16, tag="wo", name="wo_t")
                nc.sync.dma_start(out=wo_t[:], in_=wo_d)

                def xsl(q, ch, lo, hi):  # x slice for quarter q, chunk ch
                    if q == 0:
                        t_ = xq0a if ch < 4 else xq0b
                        return t_[:, (ch % 4) * 512 + lo : (ch % 4) * 512 + hi]
                    return xqt[q][:, ch * 512 + lo : ch * 512 + hi]

                ring = pring.tile([128, 2048], f32, tag="ring", name="ring")

                # ---- Q/K projections; eviction = DVE tensor_scalar (+bias) ----
                def project(pr, is_k, dst, q):
                    base = pr * 2048 + (1024 if is_k else 0)
                    ps = psw.tile([128, 512], f32, tag="pw", name=f"ps_{pr}{is_k}{q}")
                    for ch in range(NDC):
                        nc.tensor.matmul(
                            ps[:],
                            wqk_t[:, base + ch * 128 : base + (ch + 1) * 128],
                            xsl(q, ch, 0, 512),
                            start=(ch == 0),
                            stop=(ch == NDC - 1),
                        )
                    nc.vector.tensor_scalar_add(
                        dst[:, q * 512 : (q + 1) * 512],
                        ps[:],
                        bia_t[:, is_k * 2 + pr : is_k * 2 + pr + 1],
                    )

                def rope(t_, name, q):  # rope one 512-col t-quarter in place
                    # tbl layout is quarter-interleaved: [cos_q | sin_q] x 4
                    sl = slice(q * 512, (q + 1) * 512)
                    cos_q = tbl_t[:, q * 1024 : q * 1024 + 512]
                    sin_q = tbl_t[:, q * 1024 + 512 : (q + 1) * 1024]
                    sh = shpool.tile([128, 512], bf16, tag="sh", name=f"sh_{name}{q}")
                    nc.vector.stream_shuffle(
                        sh.bitcast(u32)[:], t_.bitcast(u32)[:, q * 256 : (q + 1) * 256],
                        _SHUF_MASK,
                    )
                    nc.vector.tensor_tensor(
                        out=t_[:, sl], in0=t_[:, sl], in1=cos_q, op=MUL
                    )
                    nc.vector.tensor_tensor(out=sh[:], in0=sh[:], in1=sin_q, op=MUL)
                    nc.vector.tensor_tensor(
                        out=t_[:, sl], in0=t_[:, sl], in1=sh[:], op=mybir.AluOpType.add
                    )

                qts, kts = [], []
                for pr in range(2):
                    qts.append(qkpool.tile([128, T], bf16, tag="qk", name=f"qt{pr}"))
                    kts.append(qkpool.tile([128, T], bf16, tag="qk", name=f"kt{pr}"))

                # ---- V projection (bf16 tiles, ones col via memset) ----
                vts = [None] * NKC

                def vproj(tk):
                    vt = vpool.tile([128, HPC * 65], bf16, tag="v", name=f"v{tk}")
                    nc.vector.memset(
                        vt.rearrange("p (h c) -> p h c", c=65)[:, :, 64:65], 1.0
                    )
                    ps = psw.tile([128, 256], f32, tag="pw", name=f"psv{tk}")
                    for ch in range(NDC):
                        nc.tensor.matmul(
                            ps[:],
                            xsl(tk // 4, ch, (tk % 4) * 128, (tk % 4) * 128 + 128),
                            wv_t[:, ch * 256 : (ch + 1) * 256],
                            start=(ch == 0),
                            stop=(ch == NDC - 1),
                        )
                    nc.vector.tensor_copy(
                        vt.rearrange("p (h c) -> p h c", c=65)[:, :, 0:64],
                        ps.rearrange("p (h c) -> p h c", c=64),
                    )
                    vts[tk] = vt

                ot0 = otpool.tile([128, T], bf16, tag="o", name="ot0")
                ot1 = otpool.tile([128, T], bf16, tag="o", name="ot1")
                ots = [ot0, ot1]

                # ---- flat attention pipeline over 128 global kc-slots ----
                # slot s -> unit u = s//16 = (pr = u//4, qt qi = u%4), kc = s%16.
                # Emission: prologue S(0..3); then slot k: [S(k+4)] [filler]
                # [PV(k)]; exp(s) is emitted right after S(s). The 4-slot
                # S-lead keeps the ACT exp stream saturated: S(k+4) only
                # ring-waits exp(k+2), never a PV.
                NSLOT = 128
                pts = [None] * NSLOT
                oaccs = {}

                def emit_S(s):
                    u, kc = s // 16, s % 16
                    pr, qi = u // 4, u % 4
                    goff = (s % 2) * 1024
                    for hh in range(2):
                        nc.tensor.matmul(
                            ring[:, goff + hh * 512 : goff + (hh + 1) * 512],
                            kts[pr][hh * 64 : hh * 64 + 64, kc * 128 : (kc + 1) * 128],
                            qts[pr][hh * 64 : hh * 64 + 64, qi * 512 : qi * 512 + 512],
                            start=True,
                            stop=True,
                        )
                    # softmax scale is baked into Wq host-side; plain exp here
                    pt = ppool.tile([128, 1024], bf16, tag="p", name=f"p_{s}")
                    nc.scalar.activation(
                        pt[:], ring[:, goff : goff + 1024], Exp, bias=0.0, scale=1.0
                    )
                    pts[s] = pt

                def emit_PV(s):
                    u, kc = s // 16, s % 16
                    pr, qi = u // 4, u % 4
                    if kc == 0:
                        oaccs[u] = [
                            pso.tile([65, 512], f32, tag="oa", name=f"o_{u}_{hh}")
                            for hh in range(2)
                        ]
                    for hh in range(2):
                        nc.tensor.matmul(
                            oaccs[u][hh][:],
                            vts[kc][:, (2 * pr + hh) * 65 : (2 * pr + hh + 1) * 65],
                            pts[s][:, hh * 512 : (hh + 1) * 512],
                            start=(kc == 0),
                            stop=(kc == NKC - 1),
                        )
                    pts[s] = None

                # raw O eviction: one fast DVE copy per hh frees the oacc
                # PSUM bank ~0.4us after the unit's last PV; the normalize
                # chain then runs entirely in SBUF, off the critical path.
                raws = {}

                def raw_copy(u):
                    raw = rzpool.tile([64, 2048], f32, tag="rz", name=f"raw_{u}")
                    # Z rows land in a separate partition-base-0 tile: the
                    # custom DVE reciprocal requires base-0 operands
                    zr = rzpool.tile([1, 1024], f32, tag="zr", name=f"zr_{u}")
                    for hh in range(2):
                        nc.vector.tensor_copy(
                            raw[:, hh * 512 : (hh + 1) * 512], oaccs[u][hh][0:64, :]
                        )
                        nc.vector.tensor_copy(
                            zr[0:1, hh * 512 : (hh + 1) * 512], oaccs[u][hh][64:65, :]
                        )
                    raws[u] = (raw, zr)

                def normalize(u):
                    pr, qi = u // 4, u % 4
                    qs = qi * 512
                    raw, zr = raws[u]
                    for hh in range(2):
                        hs = slice(hh * 512, (hh + 1) * 512)
                        rz = rbpool.tile([1, 512], f32, tag="rz", name=f"rzz_{u}_{hh}")
                        nc.vector.reciprocal_approx_fast(rz[0:1, :], zr[0:1, hs])
                        rc = rbpool.tile([64, 512], f32, tag="rb", name=f"rc_{u}_{hh}")
                        nc.gpsimd.partition_broadcast(
                            rc[:, :], rz[0:1, :], channels=64
                        )
                        nc.vector.tensor_tensor(
                            out=ots[pr][hh * 64 : hh * 64 + 64, qs : qs + 512],
                            in0=raw[0:64, hs],
                            in1=rc[:, :],
                            op=MUL,
                        )

                # wo sub-unit j of qt qi: t-chunk tt = 4*qi+j (2 matmul pairs
                # + casts); qi<3 packs 2 chunks per [128,2048] ysb + 1 DMA,
                # the tail qt (qi=3) DMAs each chunk separately.
                ysbs = {}

                def wo_sub(qi, j):
                    tt = 4 * qi + j
                    if j % 2 == 0:
                        ysbs[(qi, j // 2)] = ypool.tile(
                            [128, 2048], bf16, tag="y", name=f"y_{qi}_{j // 2}"
                        )
                    ysb = ysbs[(qi, j // 2)]
                    for eh in range(2):
                        # tail qt borrows the (free) oacc pool for every other
                        # accumulator so the cast evictions don't serialize it
                        pool = pso if qi == 3 and eh == 1 else psw
                        tag = "oa" if pool is pso else "pw"
                        yps = pool.tile([128, 512], f32, tag=tag, name=f"yp_{tt}_{eh}")
                        for r in range(2):
                            nc.tensor.matmul(
                                yps[:],
                                ots[r][:, tt * 128 : (tt + 1) * 128],
                                wo_t[:, r * 1024 + eh * 512 : r * 1024 + (eh + 1) * 512],
                                start=(r == 0),
                                stop=(r == 1),
                            )
                        nc.vector.tensor_copy(
                            ysb[:, (j % 2) * 1024 + eh * 512 : (j % 2) * 1024 + (eh + 1) * 512],
                            yps[:],
                        )
                    if qi == 3:
                        eng = nc.sync if j % 2 == 0 else nc.scalar
                        eng.dma_start(
                            out=y_r[:, tt : tt + 1, :],
                            in_=ysb.rearrange("p (b e) -> p b e", e=1024)[
                                :, j % 2 : j % 2 + 1, :
                            ],
                        )
                    elif j % 2 == 1:
                        nc.sync.dma_start(
                            out=y_r[:, 4 * qi + j - 1 : 4 * qi + j + 1, :],
                            in_=ysb.rearrange("p (b e) -> p b e", e=1024),
                        )

                # ---- filler schedule: slot -> list of thunks; fillers are
                # emitted BEFORE the slot's S so late Q/K quarters can feed
                # the S stream just-in-time ----
                fillers = {}

                def add_filler(slot, fn):
                    fillers.setdefault(slot, []).append(fn)

                def qk_part(q, is_k):  # one Q0/K0 proj quarter + its rope
                    def fn():
                        t_ = kts[0] if is_k else qts[0]
                        project(0, is_k, t_, q)
                        rope(t_, "k0" if is_k else "q0", q)

                    return fn

                # quarter q roped before S(4q) at slot 4q-4; split Q/K halves
                # across two slots to limit per-slot PE insertion
                for q, (sq, sk) in ((1, (0, 0)), (2, (2, 3)), (3, (6, 7))):
                    add_filler(sq, qk_part(q, 0))
                    add_filler(sk, qk_part(q, 1))
                vslots = [1, 4, 4, 5, 8, 8, 9, 10, 11, 12, 13, 14]  # V4..V15
                for i, sl in enumerate(vslots):
                    add_filler(sl, (lambda tk: lambda: vproj(tk))(4 + i))
                pj = 0  # pr1 projections + quarter-ropes woven into units 2-3
                for is_k in range(2):
                    for q in range(4):
                        add_filler(
                            32 + 2 * pj,
                            (lambda ik, qq: lambda: project(
                                1, ik, kts[1] if ik else qts[1], qq
                            ))(is_k, q),
                        )
                        add_filler(
                            33 + 2 * pj,
                            (lambda ik, qq: lambda: rope(
                                kts[1] if ik else qts[1], "k1" if ik else "q1", qq
                            ))(is_k, q),
                        )
                        pj += 1
                # wo(qi) woven into unit 5+qi AFTER each slot's S/PV so it
                # never delays the exp stream; wo(3) is the tail
                postfill = {}
                for qi in range(3):
                    for j in range(4):
                        postfill.setdefault(
                            (5 + qi) * 16 + 2 + 3 * j, []
                        ).append((lambda a, b: lambda: wo_sub(a, b))(qi, j))

                # ---- prologue: warm the PE p-state on the (tiny, landed)
                # bias tile while x/weights stream in, then quarter 0 ----
                warm = psw.tile([128, 512], f32, tag="pw", name="warm")
                for w in range(24):
                    nc.tensor.matmul(
                        warm[0:4, 0:4],
                        bia_t.bitcast(bf16)[:, 0:4],
                        bia_t.bitcast(bf16)[:, 0:4],
                        start=True,
                        stop=True,
                    )
                qk_part(0, 0)()
                qk_part(0, 1)()
                for s in range(4):
                    emit_S(s)
                for tk in range(4):
                    vproj(tk)

                # ---- steady state; PV lags S-emission by 5 slots ----
                for k in range(NSLOT + 1):
                    if k > 16 and k % 16 == 1:
                        normalize(k // 16 - 1)
                    for fn in fillers.get(k, ()):
                        fn()
                    if k + 4 < NSLOT:
                        emit_S(k + 4)
                    if k > 0:
                        emit_PV(k - 1)
                    if k > 0 and k % 16 == 0 and k // 16 - 1 != 7:
                        raw_copy(k // 16 - 1)
                    for fn in postfill.get(k, ()):
                        fn()

                # ---- tail: wo for the last qt; its normalize reads the
                # oacc PSUM directly (skips the raw-copy hop) ----
                for hh in range(2):
                    rz = rzpool.tile([1, 1024], f32, tag="zr", name=f"rzt_{hh}")
                    nc.vector.tensor_copy(rz[0:1, 0:512], oaccs[7][hh][64:65, :])
                    nc.vector.reciprocal_approx_fast(
                        rz[0:1, 512:1024], rz[0:1, 0:512]
                    )
                    rb = rbpool.tile([64, 512], f32, tag="rb", name=f"rbt_{hh}")
                    nc.gpsimd.partition_broadcast(
                        rb[:, :], rz[0:1, 512:1024], channels=64
                    )
                    nc.vector.tensor_tensor(
                        out=ots[1][hh * 64 : hh * 64 + 64, 3 * 512 : 4 * 512],
                        in0=oaccs[7][hh][0:64, :],
                        in1=rb[:, :],
                        op=MUL,
                    )
                for j in range(4):
                    wo_sub(3, j)

            if iters:
                import concourse.mybir as _mb

                with tc.For_i(
                    0,
                    iters,
                    1,
                    hint_engines=(
                        _mb.EngineType.PE,
                        _mb.EngineType.Activation,
                        _mb.EngineType.DVE,
                        _mb.EngineType.SP,
                        _mb.EngineType.Pool,
                    ),
                    staggered_reset=True,
                ) as _iv:
                    body()
            else:
                body()

    nc.compile()
    return nc


def _host_inputs(x, wq_w, wq_b, wk_w, wk_b, wv_w, wv_b, wo_w, wo_b):
    """Build the 8 per-core input maps (all host-side slicing/packing)."""
    f = np.float32
    x = np.asarray(x, f)
    wq_w = np.asarray(wq_w, f)
    wk_w = np.asarray(wk_w, f)
    wv_w = np.asarray(wv_w, f)
    wo_w = np.asarray(wo_w, f)
    wq_b = np.asarray(wq_b, f)
    wk_b = np.asarray(wk_b, f)
    wv_b = np.asarray(wv_b, f)
    wo_b = np.asarray(wo_b, f)

    def chunkpack(a, ncol):  # [1024, ncol] -> [128, 8*ncol] (D-chunk packed)
        return np.ascontiguousarray(
            a.reshape(NDC, 128, ncol).transpose(1, 0, 2).reshape(128, NDC * ncol)
        )

    # RoPE tables in fp32, mirroring the reference formulas; stored bf16.
    pos = np.arange(T, dtype=f)[:, None]
    idx = np.arange(HALF, dtype=f)[None, :]
    inv_freq = (f(1.0) / (f(ROPE_BASE) ** (idx / f(HALF)))).astype(f)
    ang = pos * inv_freq  # [T, 32]
    cosv, sinv = np.cos(ang).astype(f), np.sin(ang).astype(f)
    cos64 = np.repeat(cosv.T, 2, axis=0)  # [64, T]
    sin64 = np.repeat(sinv.T, 2, axis=0)
    sin64[0::2] *= -1  # rows 2j: -sin, rows 2j+1: +sin
    cos128 = np.tile(cos64, (2, 1))
    sin128 = np.tile(sin64, (2, 1))
    # quarter-interleaved: [cos_q | sin_q] per 512-col t-quarter
    tbl = np.ascontiguousarray(
        np.concatenate(
            [
                np.concatenate(
                    [cos128[:, q * 512 : (q + 1) * 512], sin128[:, q * 512 : (q + 1) * 512]],
                    axis=1,
                )
                for q in range(4)
            ],
            axis=1,
        ).astype(BF16)
    )

    perm64 = np.empty(64, np.int64)
    perm64[0::2] = np.arange(32)
    perm64[1::2] = np.arange(32) + 32

    # x[b]^T quarter-packed: quarter q holds all 8 D-chunks for t in
    # [512q, 512(q+1)): [128, 8ch x 512t]
    xqp = []
    for b in range(B):
        xt = x[b].T.reshape(NDC, 128, 4, 512)  # [ch, p, q, t]
        xqp.append(
            np.ascontiguousarray(
                xt.transpose(2, 1, 0, 3).reshape(4, 128, NDC * 512)
                .transpose(1, 0, 2).reshape(128, 4 * NDC * 512)
            ).astype(BF16)
        )

    in_maps = []
    for c in range(N_CORES):
        b, g = c // 4, c % 4
        heads = np.arange(4 * g, 4 * g + 4)
        v_cols = np.concatenate([np.arange(h * 64, (h + 1) * 64) for h in heads])
        # softmax scale baked into Wq/qb so the exp runs with scale=1.0
        wqk_parts, bia_cols = [], []
        for w_, b_, sc in ((wq_w, wq_b, np.float32(SCALE)), (wk_w, wk_b, np.float32(1.0))):
            for pr in range(2):
                prheads = heads[2 * pr : 2 * pr + 2]
                cols = np.concatenate([h * 64 + perm64 for h in prheads])
                wqk_parts.append((pr, chunkpack(w_[:, cols] * sc, 128)))
                bia_cols.append((pr, b_[cols] * sc))
        # layout: wq0 | wk0 | wq1 | wk1  (each [128, 1024])
        order = [0, 2, 1, 3]  # indices into wqk_parts (built q0,q1,k0,k1)
        wqk = np.concatenate([wqk_parts[i][1] for i in order], axis=1).astype(BF16)
        # bias cols: qb0, qb1, kb0, kb1
        bia = np.stack(
            [bia_cols[0][1], bia_cols[1][1], bia_cols[2][1], bia_cols[3][1]], axis=1
        ).astype(f)
        wvp = chunkpack(wv_w[:, v_cols], 256).astype(BF16)
        wop = np.ascontiguousarray(
            wo_w[v_cols, :]
            .reshape(2, 128, D)
            .transpose(1, 0, 2)
            .reshape(128, 2 * D)
            .astype(BF16)
        )
        in_maps.append(
            {
                "xqp": xqp[b],
                "wqk": np.ascontiguousarray(wqk),
                "wvp": np.ascontiguousarray(wvp),
                "tbl": tbl,
                "bia": np.ascontiguousarray(bia),
                "wop": wop,
            }
        )

    beff = (
        wo_b.astype(np.float64) + wv_b.astype(np.float64) @ wo_w.astype(np.float64)
    ).astype(f)
    return in_maps, beff


def kernel(x, wq_w, wq_b, wk_w, wk_b, wv_w, wv_b, wo_w, wo_b):
    from concourse import bass2jax

    in_maps, beff = _host_inputs(
        x, wq_w, wq_b, wk_w, wk_b, wv_w, wv_b, wo_w, wo_b
    )
    if "nc" not in _ctx:
        _ctx["nc"] = _build_nc(0)
    res = bass2jax.run_bass_via_pjrt(_ctx["nc"], in_maps, n_cores=N_CORES)
    y = np.empty((B, T, D), np.float32)
    for b in range(B):
        acc = np.asarray(res[4 * b]["y"], np.float32)
        for g in range(1, 4):
            acc += np.asarray(res[4 * b + g]["y"], np.float32)
        y[b] = acc + beff[None, :]
    return y
